# revision 4
# baseline (speedup 1.0000x reference)
"""Trainium2 Bass kernel for nn_Compressor (sparse_attention block compressor).

Math (reference):
  proj = x @ [W_kv; W_gate]^T            # [b*s, 2048]
  kv   = proj[:, :1024] + ape[s%4]       # blockwise (RATIO=4) abs-pos bias
  sc   = proj[:, 1024:]
  window(blk) = {prev blk rows, ch 0:512} + {cur blk rows, ch 512:1024}
  pooled[blk, c] = softmax-gated channelwise pool over the 8-entry window
  out = (RMSNorm(pooled) -> rope on ch 448:512) @ H  (512x512 Hadamard)

Distribution: 8 cores, data-parallel over (batch, seq-half). Each core owns
2048 seq rows = 512 blocks; the 1-block halo is handled by shifting the
matmul rhs window by 4 rows (xs input carries 16 halo rows).

Key implementation tricks:
  * x^T in bf16 obtained host-side by truncating f32 to the hi-16 planes.
  * Projections: W^T tiles stationary (lhsT), x^T moving -> PSUM layout
    [channels(part), m(free)], so the whole softmax pooling is free-axis
    DVE/ACT work and the halo is a free-axis slice offset.
  * Softmax without max-subtraction (scores are ~N(0,1.3); fp32 exp cannot
    overflow; block-0 masking is a 0/1 multiply on exp with a per-core mask).
  * Per-d-chunk x^T tiles and quarter-tile weight DMAs give fine-grained
    dependencies, so the PE starts ~1us into the kernel instead of waiting
    for whole-tile DMAs.
  * RMSNorm channel reduction via tiny accumulating matmuls with
    lhsT=pooled^2 chunks -> var lands with partition=block, matching the
    Hadamard output layout; scale is applied per-partition by ACT after the
    final Hadamard matmul (everything in between is linear).
  * Hadamard matmuls in fp16 with H rows +-1 (exact); the 1/sqrt(512)
    normalization folds into the RMSNorm scale. 1 cycle/row vs 4 for fp32.
  * j-groups processed in order (3,0,1,2) so the rope fix-up on chunk 3 is
    off the end-of-kernel critical path.
"""

import os
import numpy as np
import ml_dtypes

import concourse.bass as bass
import concourse.bacc as bacc
import concourse.mybir as mybir
from concourse.tile import TileContext
from concourse.bass_utils import run_bass_kernel_spmd

BF16 = ml_dtypes.bfloat16
F32 = mybir.dt.float32
F16 = mybir.dt.float16
BF = mybir.dt.bfloat16

N_CORES = 8
DIM = 4096
OCH = 2048          # kv 1024 + gate 1024
ROWS = 2048         # own rows per core
XS_ROWS = 2064      # 16 halo/pad rows + 2048
MCH = 4             # m-chunks per core
MROWS = 512         # rows per m-chunk
NBLK = 128          # blocks per m-chunk
DCH = 32            # d chunks of 128
WSUB = 4            # weight sub-DMAs per o-chunk (8 d-chunks each)
# o-chunks 0..3 kv-first(prev), 4..7 kv-second(cur), 8..11 sc-first, 12..15 sc-second
FIRST_HALF = (0, 1, 2, 3, 8, 9, 10, 11)

_CACHE = {}


def _build():
    nc = bacc.Bacc("TRN2", target_bir_lowering=False, debug=False,
                   num_devices=N_CORES)
    xs = nc.dram_tensor("xs", [DIM, XS_ROWS], BF, kind="ExternalInput")
    wp = nc.dram_tensor("wp", [16, 128, DCH, 128], BF, kind="ExternalInput")
    ape_d = nc.dram_tensor("ape_t", [128, 32], F32, kind="ExternalInput")
    cos_d = nc.dram_tensor("cos_t", [128, 512], F16, kind="ExternalInput")
    sin_d = nc.dram_tensor("sin_t", [128, 512], F16, kind="ExternalInput")
    psw_d = nc.dram_tensor("psw", [128, 128], F16, kind="ExternalInput")
    h_d = nc.dram_tensor("hmat", [128, 4, 512], F16, kind="ExternalInput")
    zmask_d = nc.dram_tensor("zmask", [128, 1], F32, kind="ExternalInput")
    out_d = nc.dram_tensor("out", [4 * NBLK, 512], F32, kind="ExternalOutput")

    X = mybir.AxisListType.X

    with TileContext(nc) as tc:
        with (
            tc.tile_pool(name="const", bufs=1) as constp,
            tc.tile_pool(name="xt", bufs=2) as xtp,
            tc.tile_pool(name="wt", bufs=2) as wtp,
            tc.tile_pool(name="sb", bufs=2) as sbp,
            tc.tile_pool(name="pl", bufs=2) as plp,
            tc.tile_pool(name="sm", bufs=2) as smp,
            tc.tile_pool(name="osb", bufs=2) as outp,
            tc.tile_pool(name="proj", bufs=4, space="PSUM") as projp,
            tc.tile_pool(name="had", bufs=2, space="PSUM") as hadp,
            tc.tile_pool(name="aux", bufs=1, space="PSUM") as auxp,
        ):
            # ---- constants (scalar queue: don't block the weight queue) ----
            ape_sb = constp.tile([128, 32], F32, tag="ape")
            nc.scalar.dma_start(out=ape_sb[:], in_=ape_d[:, :])
            cos_sb = constp.tile([128, 512], F16, tag="cos")
            nc.scalar.dma_start(out=cos_sb[:], in_=cos_d[:, :])
            sin_sb = constp.tile([128, 512], F16, tag="sin")
            nc.scalar.dma_start(out=sin_sb[:], in_=sin_d[:, :])
            psw_sb = constp.tile([128, 128], F16, tag="psw")
            nc.scalar.dma_start(out=psw_sb[:], in_=psw_d[:, :])
            h_sb = constp.tile([128, 4, 512], F16, tag="h")
            nc.scalar.dma_start(out=h_sb[:], in_=h_d[:, :, :])
            zmask_sb = constp.tile([128, 1], F32, tag="zmask")
            nc.scalar.dma_start(out=zmask_sb[:], in_=zmask_d[:, :])
            ones_sb = constp.tile([128, 1], F32, tag="ones")
            nc.vector.memset(ones_sb[:], 1.0)
            eps_sb = constp.tile([128, 1], F32, tag="eps")
            nc.vector.memset(eps_sb[:], 512e-6)

            for mch in range(MCH):
                r0 = MROWS * mch
                # ---- x^T tiles, one per 128-d chunk: [128(d), 528 m] bf16.
                # xs is the host-pre-transposed trunc-bf16 x^T (slot s <->
                # own row r0 + s - 16; slots 12..15 = halo rows r0-4..r0-1).
                xts = []
                for c in range(DCH):
                    xt = xtp.tile([128, 528], BF, tag=f"xt{c}")
                    nc.sync.dma_start(
                        out=xt[:],
                        in_=xs[128 * c:128 * (c + 1), r0:r0 + 528],
                    )
                    xts.append(xt)

                pooled = plp.tile([128, 4, NBLK], F16, tag="pooled")
                sq = plp.tile([128, 4, NBLK], F32, tag="sq")
                # j order: chunk 3 first so its rope fix-up overlaps later
                # groups instead of sitting on the end-of-kernel tail.
                for j in (3, 0, 1, 2):
                    group = {}
                    for t, oc in enumerate((j, j + 4, j + 8, j + 12)):
                        ws = []
                        for s in range(WSUB):
                            w = wtp.tile([128, 8, 128], BF, tag=f"w{t}s{s}")
                            nc.gpsimd.dma_start(
                                out=w[:], in_=wp[oc, :, 8 * s:8 * (s + 1), :])
                            ws.append(w)
                        ps = projp.tile([128, MROWS], F32, tag="proj")
                        off = 12 if oc in FIRST_HALF else 16
                        for d in range(DCH):
                            nc.tensor.matmul(
                                ps[:],
                                lhsT=ws[d // 8][:, d % 8, :],
                                rhs=xts[d][:, off:off + MROWS],
                                start=(d == 0),
                                stop=(d == DCH - 1),
                            )
                        if oc < 8:
                            # kv chunk: PSUM -> SBUF with ape bias added
                            kv = sbp.tile([128, MROWS], F32, tag=f"kv{t}")
                            a = oc  # ape chunk = kv o-chunk (0..7)
                            ape_ap = (ape_sb[:, 4 * a:4 * a + 4]
                                      .unsqueeze(1).to_broadcast((128, NBLK, 4)))
                            nc.vector.tensor_add(
                                kv[:].rearrange("p (b r) -> p b r", r=4),
                                ps[:].rearrange("p (b r) -> p b r", r=4),
                                ape_ap,
                            )
                            group[f"kv{t}"] = kv
                        else:
                            # score chunk: e = exp(psum) straight to SBUF
                            e = sbp.tile([128, MROWS], F32, tag=f"e{t}")
                            nc.scalar.activation(
                                e[:], ps[:], mybir.ActivationFunctionType.Exp)
                            if mch == 0 and oc < 12:
                                # block-0 of even cores: zero the 4 prev-window
                                # weights (zmask = 0 even / 1 odd)
                                nc.vector.tensor_scalar_mul(
                                    e[:, 0:4], e[:, 0:4], zmask_sb[:, 0:1])
                            group[f"e{t}"] = e

                    kv1, kv2 = group["kv0"], group["kv1"]
                    e1, e2 = group["e2"], group["e3"]

                    def g4(tile_ap):
                        return tile_ap.rearrange("p (b r) -> p b r", r=4)

                    s1 = smp.tile([128, NBLK], F32, tag="s1")
                    nc.vector.reduce_sum(s1[:], g4(e1[:]), axis=X)
                    s2 = smp.tile([128, NBLK], F32, tag="s2")
                    nc.vector.reduce_sum(s2[:], g4(e2[:]), axis=X)
                    ssum = smp.tile([128, NBLK], F32, tag="ssum")
                    nc.vector.tensor_add(ssum[:], s1[:], s2[:])

                    pm = sbp.tile([128, MROWS], F32, tag="pm")
                    nc.vector.tensor_mul(pm[:], e1[:], kv1[:])
                    q1 = smp.tile([128, NBLK], F32, tag="q1")
                    nc.vector.reduce_sum(q1[:], g4(pm[:]), axis=X)
                    pm2 = sbp.tile([128, MROWS], F32, tag="pm2")
                    nc.vector.tensor_mul(pm2[:], e2[:], kv2[:])
                    q2 = smp.tile([128, NBLK], F32, tag="q2")
                    nc.vector.reduce_sum(q2[:], g4(pm2[:]), axis=X)
                    qsum = smp.tile([128, NBLK], F32, tag="qsum")
                    nc.vector.tensor_add(qsum[:], q1[:], q2[:])

                    rinv = smp.tile([128, NBLK], F32, tag="rinv")
                    nc.vector.reciprocal(rinv[:], ssum[:])
                    nc.vector.tensor_mul(pooled[:, j, :], qsum[:], rinv[:])
                    # squared copy for the RMSNorm variance (pre-rope, which
                    # matches the reference: rope is norm-preserving and the
                    # norm scale is applied at the very end)
                    nc.scalar.activation(
                        sq[:, j, :], pooled[:, j, :],
                        mybir.ActivationFunctionType.Square)

                    if j == 3:
                        # rope on chunk 3 (ch 384..511; rows 64.. are rope),
                        # right after its pooling so it overlaps group 0
                        sw_ps = auxp.tile([128, NBLK], F32, tag="swap")
                        nc.tensor.matmul(sw_ps[:], lhsT=psw_sb[:],
                                         rhs=pooled[:, 3, :],
                                         start=True, stop=True)
                        cslice = cos_sb[:, mch * NBLK:(mch + 1) * NBLK]
                        sslice = sin_sb[:, mch * NBLK:(mch + 1) * NBLK]
                        tmpc = smp.tile([128, NBLK], F16, tag="tmpc")
                        nc.vector.tensor_mul(tmpc[:], pooled[:, 3, :], cslice)
                        tmps = smp.tile([128, NBLK], F16, tag="tmps")
                        nc.vector.tensor_mul(tmps[:], sw_ps[:], sslice)
                        nc.vector.tensor_add(pooled[:, 3, :], tmpc[:], tmps[:])

                # ---- RMSNorm stats: var[blk] via accumulating matmuls with
                # lhsT=sq chunks -> PSUM [128(blk), 1]
                var_ps = auxp.tile([128, 1], F32, tag="var")
                for j in range(4):
                    nc.tensor.matmul(var_ps[:], lhsT=sq[:, j, :],
                                     rhs=ones_sb[:, 0:1],
                                     start=(j == 0), stop=(j == 3))
                sd_col = smp.tile([128, 1], F32, tag="sd_col")
                # sd = sqrt(var + 512*eps) = sqrt(512) * sqrt(var/512 + eps);
                # the extra 1/sqrt(512) folds the Hadamard normalization (H
                # rows are +-1 on device)
                nc.scalar.activation(sd_col[:], var_ps[:],
                                     mybir.ActivationFunctionType.Sqrt,
                                     scale=1.0, bias=eps_sb[:, 0:1])
                scale_col = smp.tile([128, 1], F32, tag="scale_col")
                nc.vector.reciprocal(scale_col[:], sd_col[:])

                # ---- Hadamard: out[blk, c'] = sum_c pooled[c, blk] H[c, c']
                # (fp16 operands: 1 cycle/row vs 4 for fp32; H rows are +-1, exact)
                had_ps = hadp.tile([128, 512], F32, tag="had")
                for j in range(4):
                    nc.tensor.matmul(had_ps[:],
                                     lhsT=pooled[:, j, :],
                                     rhs=h_sb[:, j, :],
                                     start=(j == 0), stop=(j == 3))
                out_sb = outp.tile([128, 512], F32, tag="out")
                nc.scalar.activation(out_sb[:], had_ps[:],
                                     mybir.ActivationFunctionType.Copy,
                                     scale=scale_col[:, 0:1])
                nc.gpsimd.dma_start(
                    out=out_d[mch * NBLK:(mch + 1) * NBLK, :], in_=out_sb[:])
    nc.compile()
    return nc


def _prep_shared(W_kv, W_gate, ape, norm_w, H):
    W = np.concatenate([W_kv, W_gate], axis=0).astype(np.float32)  # [2048, 4096]
    Wb = W.astype(BF16)
    wp = np.ascontiguousarray(
        Wb.T.reshape(DCH, 128, 16, 128).transpose(2, 1, 0, 3))  # [16,128,32,128]
    ape_t = np.ascontiguousarray(
        ape.astype(np.float32).T.reshape(8, 128, 4).transpose(1, 0, 2)
    ).reshape(128, 32)
    psw = np.zeros((128, 128), np.float16)
    idx = np.arange(64)
    psw[idx, idx] = 1.0
    k2 = np.arange(0, 64, 2)
    psw[64 + k2 + 1, 64 + k2] = 1.0
    psw[64 + k2, 64 + k2 + 1] = 1.0
    hm = np.ascontiguousarray(
        (norm_w.astype(np.float32)[:, None] * H.astype(np.float32)
         * np.sqrt(512.0, dtype=np.float32))
        .reshape(4, 128, 512).transpose(1, 0, 2)).astype(np.float16)
    return wp, ape_t, psw, hm


def _hadamard(n):
    h = np.array([[1.0]], dtype=np.float32)
    while h.shape[0] < n:
        h = np.block([[h, h], [h, -h]])
    return (h / np.sqrt(n)).astype(np.float32)


def _make_in_maps(x, W_kv, W_gate, ape, norm_w, freqs_cis):
    b, s, _ = x.shape
    H = _hadamard(512)
    wp, ape_t, psw, hm = _prep_shared(W_kv, W_gate, ape, norm_w, H)

    # truncate-to-bf16 (hi-16 planes of the f32 words) and transpose once
    xh = x.reshape(b * s, DIM).view(BF16)[:, 1::2]
    xT = np.ascontiguousarray(xh.T)  # [4096, 16384]
    fr = freqs_cis[:, :, 0]  # [nb, 32]
    fi = freqs_cis[:, :, 1]

    in_maps = []
    for c in range(N_CORES):
        batch, half = c // 2, c % 2
        R0 = batch * s + half * ROWS
        xs = np.zeros((DIM, XS_ROWS), BF16)
        xs[:, 16:] = xT[:, R0:R0 + ROWS]
        if half == 1:
            xs[:, :16] = xT[:, R0 - 16:R0]

        g0 = half * 512
        bi = np.arange(g0, g0 + 512)
        cos_t = np.zeros((128, 512), np.float16)
        cos_t[:64] = 1.0
        cos_t[64:] = np.repeat(fr[bi].T, 2, axis=0).astype(np.float16)
        sin_t = np.zeros((128, 512), np.float16)
        st = np.repeat(fi[bi].T, 2, axis=0)
        st[0::2] *= -1.0
        sin_t[64:] = st.astype(np.float16)

        zmask = np.full((128, 1), 0.0 if half == 0 else 1.0, np.float32)
        in_maps.append({
            "xs": xs, "wp": wp, "ape_t": ape_t,
            "cos_t": cos_t, "sin_t": sin_t, "psw": psw,
            "hmat": hm, "zmask": zmask,
        })
    return in_maps


def kernel(x, W_kv, W_gate, ape, norm_w, freqs_cis, start_pos=0):
    x = np.asarray(x, dtype=np.float32)
    W_kv = np.asarray(W_kv, dtype=np.float32)
    W_gate = np.asarray(W_gate, dtype=np.float32)
    ape = np.asarray(ape, dtype=np.float32)
    norm_w = np.asarray(norm_w, dtype=np.float32)
    freqs_cis = np.asarray(freqs_cis, dtype=np.float32)

    b, s, _ = x.shape
    nb = s // 4
    assert (b, s) == (4, 4096), (b, s)

    if "nc" not in _CACHE:
        _CACHE["nc"] = _build()
    nc = _CACHE["nc"]

    in_maps = _make_in_maps(x, W_kv, W_gate, ape, norm_w, freqs_cis)

    trace = os.environ.get("KERNEL_TRACE", "") not in ("", "0")
    res = run_bass_kernel_spmd(nc, in_maps, core_ids=list(range(N_CORES)),
                               trace=trace)
    kernel.last_results = res
    out = np.concatenate([res.results[c]["out"] for c in range(N_CORES)], axis=0)
    return np.ascontiguousarray(out.reshape(b, nb, 512))


# revision 6
# speedup vs baseline: 1.1794x; 1.1794x over previous
"""Trainium2 Bass kernel for nn_Compressor (sparse_attention block compressor).

Math (reference):
  proj = x @ [W_kv; W_gate]^T            # [b*s, 2048]
  kv   = proj[:, :1024] + ape[s%4]       # blockwise (RATIO=4) abs-pos bias
  sc   = proj[:, 1024:]
  window(blk) = {prev blk rows, ch 0:512} + {cur blk rows, ch 512:1024}
  pooled[blk, c] = softmax-gated channelwise pool over the 8-entry window
  out = (RMSNorm(pooled) -> rope on ch 448:512) @ H  (512x512 Hadamard)

Distribution: 8 cores, data-parallel over (batch, seq-half). Each core owns
2048 seq rows = 512 blocks; the 1-block halo is handled by shifting the
matmul rhs window by 4 rows (xs input carries 16 halo rows).

Key implementation tricks:
  * x^T in bf16 obtained host-side by truncating f32 to the hi-16 planes.
  * Projections: W^T tiles stationary (lhsT), x^T moving -> PSUM layout
    [channels(part), m(free)], so the whole softmax pooling is free-axis
    DVE/ACT work and the halo is a free-axis slice offset.
  * Softmax without max-subtraction (scores are ~N(0,1.3); fp32 exp cannot
    overflow; block-0 masking is a 0/1 multiply on exp with a per-core mask).
  * Per-d-chunk x^T tiles and quarter-tile weight DMAs give fine-grained
    dependencies, so the PE starts ~1us into the kernel instead of waiting
    for whole-tile DMAs.
  * RMSNorm channel reduction via tiny accumulating matmuls with
    lhsT=pooled^2 chunks -> var lands with partition=block, matching the
    Hadamard output layout; scale is applied per-partition by ACT after the
    final Hadamard matmul (everything in between is linear).
  * Hadamard matmuls in fp16 with H rows +-1 (exact); the 1/sqrt(512)
    normalization folds into the RMSNorm scale. 1 cycle/row vs 4 for fp32.
  * j-groups processed in order (3,0,1,2) so the rope fix-up on chunk 3 is
    off the end-of-kernel critical path.
"""

import os
import numpy as np
import ml_dtypes

import concourse.bass as bass
import concourse.bacc as bacc
import concourse.mybir as mybir
from concourse.tile import TileContext
from concourse.bass_utils import run_bass_kernel_spmd

BF16 = ml_dtypes.bfloat16
F32 = mybir.dt.float32
F16 = mybir.dt.float16
BF = mybir.dt.bfloat16

N_CORES = 8
DIM = 4096
OCH = 2048          # kv 1024 + gate 1024
ROWS = 2048         # own rows per core
XS_ROWS = 2064      # 16 halo/pad rows + 2048
MCH = 4             # m-chunks per core
MROWS = 512         # rows per m-chunk
NBLK = 128          # blocks per m-chunk
DCH = 32            # d chunks of 128
WSUB = 4            # weight sub-DMAs per o-chunk (8 d-chunks each)
# o-chunks 0..3 kv-first(prev), 4..7 kv-second(cur), 8..11 sc-first, 12..15 sc-second
FIRST_HALF = (0, 1, 2, 3, 8, 9, 10, 11)

_CACHE = {}


def _build():
    nc = bacc.Bacc("TRN2", target_bir_lowering=False, debug=False,
                   num_devices=N_CORES)
    xs = nc.dram_tensor("xs", [DIM, XS_ROWS], BF, kind="ExternalInput")
    wp = nc.dram_tensor("wp", [16, 128, DCH, 128], BF, kind="ExternalInput")
    ape_d = nc.dram_tensor("ape_t", [128, 32], F32, kind="ExternalInput")
    cos_d = nc.dram_tensor("cos_t", [128, 512], F16, kind="ExternalInput")
    sin_d = nc.dram_tensor("sin_t", [128, 512], F16, kind="ExternalInput")
    psw_d = nc.dram_tensor("psw", [128, 128], F16, kind="ExternalInput")
    h_d = nc.dram_tensor("hmat", [128, 4, 512], F16, kind="ExternalInput")
    zmask_d = nc.dram_tensor("zmask", [128, 1], F32, kind="ExternalInput")
    out_d = nc.dram_tensor("out", [4 * NBLK, 512], F32, kind="ExternalOutput")

    X = mybir.AxisListType.X

    with TileContext(nc) as tc:
        with (
            tc.tile_pool(name="const", bufs=1) as constp,
            tc.tile_pool(name="xt", bufs=2) as xtp,
            tc.tile_pool(name="wt", bufs=2) as wtp,
            tc.tile_pool(name="sb", bufs=2) as sbp,
            tc.tile_pool(name="pl", bufs=2) as plp,
            tc.tile_pool(name="sm", bufs=2) as smp,
            tc.tile_pool(name="osb", bufs=2) as outp,
            tc.tile_pool(name="proj", bufs=4, space="PSUM") as projp,
            tc.tile_pool(name="had", bufs=2, space="PSUM") as hadp,
            tc.tile_pool(name="aux", bufs=1, space="PSUM") as auxp,
        ):
            # ---- constants (scalar queue: don't block the weight queue) ----
            ape_sb = constp.tile([128, 32], F32, tag="ape")
            nc.scalar.dma_start(out=ape_sb[:], in_=ape_d[:, :])
            cos_sb = constp.tile([128, 512], F16, tag="cos")
            nc.scalar.dma_start(out=cos_sb[:], in_=cos_d[:, :])
            sin_sb = constp.tile([128, 512], F16, tag="sin")
            nc.scalar.dma_start(out=sin_sb[:], in_=sin_d[:, :])
            psw_sb = constp.tile([128, 128], F16, tag="psw")
            nc.scalar.dma_start(out=psw_sb[:], in_=psw_d[:, :])
            h_sb = constp.tile([128, 4, 512], F16, tag="h")
            nc.scalar.dma_start(out=h_sb[:], in_=h_d[:, :, :])
            zmask_sb = constp.tile([128, 1], F32, tag="zmask")
            nc.scalar.dma_start(out=zmask_sb[:], in_=zmask_d[:, :])
            ones_sb = constp.tile([128, 1], F32, tag="ones")
            nc.vector.memset(ones_sb[:], 1.0)
            eps_sb = constp.tile([128, 1], F32, tag="eps")
            nc.vector.memset(eps_sb[:], 512e-6)

            for mch in range(MCH):
                r0 = MROWS * mch
                # ---- x^T tiles, one per 128-d chunk: [128(d), 528 m] bf16.
                # xs is the host-pre-transposed trunc-bf16 x^T (slot s <->
                # own row r0 + s - 16; slots 12..15 = halo rows r0-4..r0-1).
                xts = []
                for c in range(DCH):
                    xt = xtp.tile([128, 528], BF, tag=f"xt{c}")
                    nc.sync.dma_start(
                        out=xt[:],
                        in_=xs[128 * c:128 * (c + 1), r0:r0 + 528],
                    )
                    xts.append(xt)

                pooled = plp.tile([128, 4, NBLK], F16, tag="pooled")
                sq = plp.tile([128, 4, NBLK], F32, tag="sq")
                # j order: chunk 3 first so its rope fix-up overlaps later
                # groups instead of sitting on the end-of-kernel tail.
                for j in (3, 0, 1, 2):
                    group = {}
                    for t, oc in enumerate((j, j + 4, j + 8, j + 12)):
                        # one DMA per weight tile (descriptor-generation on the
                        # gpsimd DGE is the scarce resource) -- except the very
                        # first tile ever, split in 4 so the PE starts early
                        if mch == 0 and j == 3 and t == 0:
                            ws = []
                            for s in range(WSUB):
                                w = constp.tile([128, 8, 128], BF, tag=f"w0s{s}")
                                nc.gpsimd.dma_start(
                                    out=w[:], in_=wp[oc, :, 8 * s:8 * (s + 1), :])
                                ws.append(w)

                            def wslice(d, ws=ws):
                                return ws[d // 8][:, d % 8, :]
                        else:
                            w = wtp.tile([128, DCH, 128], BF, tag=f"w{t}")
                            nc.gpsimd.dma_start(out=w[:], in_=wp[oc])

                            def wslice(d, w=w):
                                return w[:, d, :]
                        ps = projp.tile([128, MROWS], F32, tag="proj")
                        off = 12 if oc in FIRST_HALF else 16
                        for d in range(DCH):
                            nc.tensor.matmul(
                                ps[:],
                                lhsT=wslice(d),
                                rhs=xts[d][:, off:off + MROWS],
                                start=(d == 0),
                                stop=(d == DCH - 1),
                            )
                        if oc < 8:
                            # kv chunk: PSUM -> SBUF with ape bias added
                            kv = sbp.tile([128, MROWS], F32, tag=f"kv{t}")
                            a = oc  # ape chunk = kv o-chunk (0..7)
                            ape_ap = (ape_sb[:, 4 * a:4 * a + 4]
                                      .unsqueeze(1).to_broadcast((128, NBLK, 4)))
                            nc.vector.tensor_add(
                                kv[:].rearrange("p (b r) -> p b r", r=4),
                                ps[:].rearrange("p (b r) -> p b r", r=4),
                                ape_ap,
                            )
                            group[f"kv{t}"] = kv
                        else:
                            # score chunk: e = exp(psum) straight to SBUF
                            e = sbp.tile([128, MROWS], F32, tag=f"e{t}")
                            nc.scalar.activation(
                                e[:], ps[:], mybir.ActivationFunctionType.Exp)
                            if mch == 0 and oc < 12:
                                # block-0 of even cores: zero the 4 prev-window
                                # weights (zmask = 0 even / 1 odd)
                                nc.vector.tensor_scalar_mul(
                                    e[:, 0:4], e[:, 0:4], zmask_sb[:, 0:1])
                            group[f"e{t}"] = e

                    kv1, kv2 = group["kv0"], group["kv1"]
                    e1, e2 = group["e2"], group["e3"]

                    def g4(tile_ap):
                        return tile_ap.rearrange("p (b r) -> p b r", r=4)

                    s1 = smp.tile([128, NBLK], F32, tag="s1")
                    nc.vector.reduce_sum(s1[:], g4(e1[:]), axis=X)
                    s2 = smp.tile([128, NBLK], F32, tag="s2")
                    nc.vector.reduce_sum(s2[:], g4(e2[:]), axis=X)
                    ssum = smp.tile([128, NBLK], F32, tag="ssum")
                    nc.vector.tensor_add(ssum[:], s1[:], s2[:])

                    pm = sbp.tile([128, MROWS], F32, tag="pm")
                    nc.vector.tensor_mul(pm[:], e1[:], kv1[:])
                    q1 = smp.tile([128, NBLK], F32, tag="q1")
                    nc.vector.reduce_sum(q1[:], g4(pm[:]), axis=X)
                    pm2 = sbp.tile([128, MROWS], F32, tag="pm2")
                    nc.vector.tensor_mul(pm2[:], e2[:], kv2[:])
                    q2 = smp.tile([128, NBLK], F32, tag="q2")
                    nc.vector.reduce_sum(q2[:], g4(pm2[:]), axis=X)
                    qsum = smp.tile([128, NBLK], F32, tag="qsum")
                    nc.vector.tensor_add(qsum[:], q1[:], q2[:])

                    rinv = smp.tile([128, NBLK], F32, tag="rinv")
                    nc.vector.reciprocal(rinv[:], ssum[:])
                    nc.vector.tensor_mul(pooled[:, j, :], qsum[:], rinv[:])
                    # squared copy for the RMSNorm variance (pre-rope, which
                    # matches the reference: rope is norm-preserving and the
                    # norm scale is applied at the very end)
                    nc.scalar.activation(
                        sq[:, j, :], pooled[:, j, :],
                        mybir.ActivationFunctionType.Square)

                    if j == 3:
                        # rope on chunk 3 (ch 384..511; rows 64.. are rope),
                        # right after its pooling so it overlaps group 0
                        sw_ps = auxp.tile([128, NBLK], F32, tag="swap")
                        nc.tensor.matmul(sw_ps[:], lhsT=psw_sb[:],
                                         rhs=pooled[:, 3, :],
                                         start=True, stop=True)
                        cslice = cos_sb[:, mch * NBLK:(mch + 1) * NBLK]
                        sslice = sin_sb[:, mch * NBLK:(mch + 1) * NBLK]
                        tmpc = smp.tile([128, NBLK], F16, tag="tmpc")
                        nc.vector.tensor_mul(tmpc[:], pooled[:, 3, :], cslice)
                        tmps = smp.tile([128, NBLK], F16, tag="tmps")
                        nc.vector.tensor_mul(tmps[:], sw_ps[:], sslice)
                        nc.vector.tensor_add(pooled[:, 3, :], tmpc[:], tmps[:])

                # ---- RMSNorm stats: var[blk] via accumulating matmuls with
                # lhsT=sq chunks -> PSUM [128(blk), 1]
                var_ps = auxp.tile([128, 1], F32, tag="var")
                for j in range(4):
                    nc.tensor.matmul(var_ps[:], lhsT=sq[:, j, :],
                                     rhs=ones_sb[:, 0:1],
                                     start=(j == 0), stop=(j == 3))
                sd_col = smp.tile([128, 1], F32, tag="sd_col")
                # sd = sqrt(var + 512*eps) = sqrt(512) * sqrt(var/512 + eps);
                # the extra 1/sqrt(512) folds the Hadamard normalization (H
                # rows are +-1 on device)
                nc.scalar.activation(sd_col[:], var_ps[:],
                                     mybir.ActivationFunctionType.Sqrt,
                                     scale=1.0, bias=eps_sb[:, 0:1])
                scale_col = smp.tile([128, 1], F32, tag="scale_col")
                nc.vector.reciprocal(scale_col[:], sd_col[:])

                # ---- Hadamard: out[blk, c'] = sum_c pooled[c, blk] H[c, c']
                # (fp16 operands: 1 cycle/row vs 4 for fp32; H rows are +-1, exact)
                had_ps = hadp.tile([128, 512], F32, tag="had")
                for j in range(4):
                    nc.tensor.matmul(had_ps[:],
                                     lhsT=pooled[:, j, :],
                                     rhs=h_sb[:, j, :],
                                     start=(j == 0), stop=(j == 3))
                out_sb = outp.tile([128, 512], F32, tag="out")
                nc.scalar.activation(out_sb[:], had_ps[:],
                                     mybir.ActivationFunctionType.Copy,
                                     scale=scale_col[:, 0:1])
                nc.gpsimd.dma_start(
                    out=out_d[mch * NBLK:(mch + 1) * NBLK, :], in_=out_sb[:])
    nc.compile()
    return nc


def _prep_shared(W_kv, W_gate, ape, norm_w, H):
    W = np.concatenate([W_kv, W_gate], axis=0).astype(np.float32)  # [2048, 4096]
    Wb = W.astype(BF16)
    wp = np.ascontiguousarray(
        Wb.T.reshape(DCH, 128, 16, 128).transpose(2, 1, 0, 3))  # [16,128,32,128]
    ape_t = np.ascontiguousarray(
        ape.astype(np.float32).T.reshape(8, 128, 4).transpose(1, 0, 2)
    ).reshape(128, 32)
    psw = np.zeros((128, 128), np.float16)
    idx = np.arange(64)
    psw[idx, idx] = 1.0
    k2 = np.arange(0, 64, 2)
    psw[64 + k2 + 1, 64 + k2] = 1.0
    psw[64 + k2, 64 + k2 + 1] = 1.0
    hm = np.ascontiguousarray(
        (norm_w.astype(np.float32)[:, None] * H.astype(np.float32)
         * np.sqrt(512.0, dtype=np.float32))
        .reshape(4, 128, 512).transpose(1, 0, 2)).astype(np.float16)
    return wp, ape_t, psw, hm


def _hadamard(n):
    h = np.array([[1.0]], dtype=np.float32)
    while h.shape[0] < n:
        h = np.block([[h, h], [h, -h]])
    return (h / np.sqrt(n)).astype(np.float32)


def _make_in_maps(x, W_kv, W_gate, ape, norm_w, freqs_cis):
    b, s, _ = x.shape
    H = _hadamard(512)
    wp, ape_t, psw, hm = _prep_shared(W_kv, W_gate, ape, norm_w, H)

    # truncate-to-bf16 (hi-16 planes of the f32 words) and transpose once
    xh = x.reshape(b * s, DIM).view(BF16)[:, 1::2]
    xT = np.ascontiguousarray(xh.T)  # [4096, 16384]
    fr = freqs_cis[:, :, 0]  # [nb, 32]
    fi = freqs_cis[:, :, 1]

    in_maps = []
    for c in range(N_CORES):
        batch, half = c // 2, c % 2
        R0 = batch * s + half * ROWS
        xs = np.zeros((DIM, XS_ROWS), BF16)
        xs[:, 16:] = xT[:, R0:R0 + ROWS]
        if half == 1:
            xs[:, :16] = xT[:, R0 - 16:R0]

        g0 = half * 512
        bi = np.arange(g0, g0 + 512)
        cos_t = np.zeros((128, 512), np.float16)
        cos_t[:64] = 1.0
        cos_t[64:] = np.repeat(fr[bi].T, 2, axis=0).astype(np.float16)
        sin_t = np.zeros((128, 512), np.float16)
        st = np.repeat(fi[bi].T, 2, axis=0)
        st[0::2] *= -1.0
        sin_t[64:] = st.astype(np.float16)

        zmask = np.full((128, 1), 0.0 if half == 0 else 1.0, np.float32)
        in_maps.append({
            "xs": xs, "wp": wp, "ape_t": ape_t,
            "cos_t": cos_t, "sin_t": sin_t, "psw": psw,
            "hmat": hm, "zmask": zmask,
        })
    return in_maps


def kernel(x, W_kv, W_gate, ape, norm_w, freqs_cis, start_pos=0):
    x = np.asarray(x, dtype=np.float32)
    W_kv = np.asarray(W_kv, dtype=np.float32)
    W_gate = np.asarray(W_gate, dtype=np.float32)
    ape = np.asarray(ape, dtype=np.float32)
    norm_w = np.asarray(norm_w, dtype=np.float32)
    freqs_cis = np.asarray(freqs_cis, dtype=np.float32)

    b, s, _ = x.shape
    nb = s // 4
    assert (b, s) == (4, 4096), (b, s)

    if "nc" not in _CACHE:
        _CACHE["nc"] = _build()
    nc = _CACHE["nc"]

    in_maps = _make_in_maps(x, W_kv, W_gate, ape, norm_w, freqs_cis)

    trace = os.environ.get("KERNEL_TRACE", "") not in ("", "0")
    res = run_bass_kernel_spmd(nc, in_maps, core_ids=list(range(N_CORES)),
                               trace=trace)
    kernel.last_results = res
    out = np.concatenate([res.results[c]["out"] for c in range(N_CORES)], axis=0)
    return np.ascontiguousarray(out.reshape(b, nb, 512))


# revision 7
# speedup vs baseline: 1.2088x; 1.0249x over previous
"""Trainium2 Bass kernel for nn_Compressor (sparse_attention block compressor).

Math (reference):
  proj = x @ [W_kv; W_gate]^T            # [b*s, 2048]
  kv   = proj[:, :1024] + ape[s%4]       # blockwise (RATIO=4) abs-pos bias
  sc   = proj[:, 1024:]
  window(blk) = {prev blk rows, ch 0:512} + {cur blk rows, ch 512:1024}
  pooled[blk, c] = softmax-gated channelwise pool over the 8-entry window
  out = (RMSNorm(pooled) -> rope on ch 448:512) @ H  (512x512 Hadamard)

Distribution: 8 cores, data-parallel over (batch, seq-half). Each core owns
2048 seq rows = 512 blocks; the 1-block halo is handled by shifting the
matmul rhs window by 4 rows (xs input carries 16 halo rows).

Key implementation tricks:
  * x^T in bf16 obtained host-side by truncating f32 to the hi-16 planes.
  * Projections: W^T tiles stationary (lhsT), x^T moving -> PSUM layout
    [channels(part), m(free)], so the whole softmax pooling is free-axis
    DVE/ACT work and the halo is a free-axis slice offset.
  * Softmax without max-subtraction (scores are ~N(0,1.3); fp32 exp cannot
    overflow; block-0 masking is a 0/1 multiply on exp with a per-core mask).
  * Per-d-chunk x^T tiles and quarter-tile weight DMAs give fine-grained
    dependencies, so the PE starts ~1us into the kernel instead of waiting
    for whole-tile DMAs.
  * RMSNorm channel reduction via tiny accumulating matmuls with
    lhsT=pooled^2 chunks -> var lands with partition=block, matching the
    Hadamard output layout; scale is applied per-partition by ACT after the
    final Hadamard matmul (everything in between is linear).
  * Hadamard matmuls in fp16 with H rows +-1 (exact); the 1/sqrt(512)
    normalization folds into the RMSNorm scale. 1 cycle/row vs 4 for fp32.
  * j-groups processed in order (3,0,1,2) so the rope fix-up on chunk 3 is
    off the end-of-kernel critical path.
"""

import os
import numpy as np
import ml_dtypes

import concourse.bass as bass
import concourse.bacc as bacc
import concourse.mybir as mybir
from concourse.tile import TileContext
from concourse.bass_utils import run_bass_kernel_spmd

BF16 = ml_dtypes.bfloat16
F32 = mybir.dt.float32
F16 = mybir.dt.float16
BF = mybir.dt.bfloat16

N_CORES = 8
DIM = 4096
OCH = 2048          # kv 1024 + gate 1024
ROWS = 2048         # own rows per core
XS_ROWS = 2064      # 16 halo/pad rows + 2048
MCH = 4             # m-chunks per core
MROWS = 512         # rows per m-chunk
NBLK = 128          # blocks per m-chunk
DCH = 32            # d chunks of 128
WSUB = 4            # weight sub-DMAs per o-chunk (8 d-chunks each)
# o-chunks 0..3 kv-first(prev), 4..7 kv-second(cur), 8..11 sc-first, 12..15 sc-second
FIRST_HALF = (0, 1, 2, 3, 8, 9, 10, 11)

_CACHE = {}


def _build():
    nc = bacc.Bacc("TRN2", target_bir_lowering=False, debug=False,
                   num_devices=N_CORES)
    xs = nc.dram_tensor("xs", [DIM, XS_ROWS], BF, kind="ExternalInput")
    wp = nc.dram_tensor("wp", [16, 128, DCH, 128], BF, kind="ExternalInput")
    ape_d = nc.dram_tensor("ape_t", [128, 32], F32, kind="ExternalInput")
    cos_d = nc.dram_tensor("cos_t", [128, 512], F16, kind="ExternalInput")
    sin_d = nc.dram_tensor("sin_t", [128, 512], F16, kind="ExternalInput")
    psw_d = nc.dram_tensor("psw", [128, 128], F16, kind="ExternalInput")
    h_d = nc.dram_tensor("hmat", [128, 4, 512], F16, kind="ExternalInput")
    zmask_d = nc.dram_tensor("zmask", [128, 1], F32, kind="ExternalInput")
    out_d = nc.dram_tensor("out", [4 * NBLK, 512], F32, kind="ExternalOutput")

    X = mybir.AxisListType.X

    with TileContext(nc) as tc:
        with (
            tc.tile_pool(name="const", bufs=1) as constp,
            tc.tile_pool(name="xt", bufs=2) as xtp,
            tc.tile_pool(name="wt", bufs=2) as wtp,
            tc.tile_pool(name="sb", bufs=2) as sbp,
            tc.tile_pool(name="pl", bufs=2) as plp,
            tc.tile_pool(name="sm", bufs=2) as smp,
            tc.tile_pool(name="osb", bufs=2) as outp,
            tc.tile_pool(name="proj", bufs=4, space="PSUM") as projp,
            tc.tile_pool(name="had", bufs=2, space="PSUM") as hadp,
            tc.tile_pool(name="aux", bufs=1, space="PSUM") as auxp,
        ):
            # ---- constants (scalar queue: don't block the weight queue) ----
            ape_sb = constp.tile([128, 32], F32, tag="ape")
            nc.scalar.dma_start(out=ape_sb[:], in_=ape_d[:, :])
            cos_sb = constp.tile([128, 512], F16, tag="cos")
            nc.scalar.dma_start(out=cos_sb[:], in_=cos_d[:, :])
            sin_sb = constp.tile([128, 512], F16, tag="sin")
            nc.scalar.dma_start(out=sin_sb[:], in_=sin_d[:, :])
            psw_sb = constp.tile([128, 128], F16, tag="psw")
            nc.scalar.dma_start(out=psw_sb[:], in_=psw_d[:, :])
            h_sb = constp.tile([128, 4, 512], F16, tag="h")
            nc.scalar.dma_start(out=h_sb[:], in_=h_d[:, :, :])
            zmask_sb = constp.tile([128, 1], F32, tag="zmask")
            nc.scalar.dma_start(out=zmask_sb[:], in_=zmask_d[:, :])
            ones_sb = constp.tile([128, 1], F32, tag="ones")
            nc.vector.memset(ones_sb[:], 1.0)
            eps_sb = constp.tile([128, 1], F32, tag="eps")
            nc.vector.memset(eps_sb[:], 512e-6)

            # first weight tile (oc=3), split in 4 so the PE starts early;
            # emitted first so it leads the single ordered input-DMA queue
            w0subs = []
            for s in range(WSUB):
                w = constp.tile([128, 8, 128], BF, tag=f"w0s{s}")
                nc.sync.dma_start(out=w[:], in_=wp[3, :, 8 * s:8 * (s + 1), :])
                w0subs.append(w)

            for mch in range(MCH):
                r0 = MROWS * mch
                # ---- x^T tiles, one per 128-d chunk: [128(d), 528 m] bf16.
                # xs is the host-pre-transposed trunc-bf16 x^T (slot s <->
                # own row r0 + s - 16; slots 12..15 = halo rows r0-4..r0-1).
                # All input DMAs share the sync queue: a single in-order queue
                # sees no descriptor-level round-robin, so the small-descriptor
                # xt chunks are not starved by the fat weight tiles.
                xts = []
                for c in range(DCH):
                    xt = xtp.tile([128, 528], BF, tag=f"xt{c}")
                    nc.sync.dma_start(
                        out=xt[:],
                        in_=xs[128 * c:128 * (c + 1), r0:r0 + 528],
                    )
                    xts.append(xt)

                pooled = plp.tile([128, 4, NBLK], F16, tag="pooled")
                sq = plp.tile([128, 4, NBLK], F32, tag="sq")
                # j order: chunk 3 first so its rope fix-up overlaps later
                # groups instead of sitting on the end-of-kernel tail.
                for j in (3, 0, 1, 2):
                    group = {}
                    for t, oc in enumerate((j, j + 4, j + 8, j + 12)):
                        # one DMA per weight tile; the very first tile was
                        # hoisted above the mch loop
                        if mch == 0 and j == 3 and t == 0:
                            def wslice(d, ws=w0subs):
                                return ws[d // 8][:, d % 8, :]
                        else:
                            w = wtp.tile([128, DCH, 128], BF, tag=f"w{t}")
                            nc.sync.dma_start(out=w[:], in_=wp[oc])

                            def wslice(d, w=w):
                                return w[:, d, :]
                        ps = projp.tile([128, MROWS], F32, tag="proj")
                        off = 12 if oc in FIRST_HALF else 16
                        for d in range(DCH):
                            nc.tensor.matmul(
                                ps[:],
                                lhsT=wslice(d),
                                rhs=xts[d][:, off:off + MROWS],
                                start=(d == 0),
                                stop=(d == DCH - 1),
                            )
                        if oc < 8:
                            # kv chunk: PSUM -> SBUF with ape bias added
                            kv = sbp.tile([128, MROWS], F32, tag=f"kv{t}")
                            a = oc  # ape chunk = kv o-chunk (0..7)
                            ape_ap = (ape_sb[:, 4 * a:4 * a + 4]
                                      .unsqueeze(1).to_broadcast((128, NBLK, 4)))
                            nc.vector.tensor_add(
                                kv[:].rearrange("p (b r) -> p b r", r=4),
                                ps[:].rearrange("p (b r) -> p b r", r=4),
                                ape_ap,
                            )
                            group[f"kv{t}"] = kv
                        else:
                            # score chunk: e = exp(psum) straight to SBUF
                            e = sbp.tile([128, MROWS], F32, tag=f"e{t}")
                            nc.scalar.activation(
                                e[:], ps[:], mybir.ActivationFunctionType.Exp)
                            if mch == 0 and oc < 12:
                                # block-0 of even cores: zero the 4 prev-window
                                # weights (zmask = 0 even / 1 odd)
                                nc.vector.tensor_scalar_mul(
                                    e[:, 0:4], e[:, 0:4], zmask_sb[:, 0:1])
                            group[f"e{t}"] = e

                    kv1, kv2 = group["kv0"], group["kv1"]
                    e1, e2 = group["e2"], group["e3"]

                    def g4(tile_ap):
                        return tile_ap.rearrange("p (b r) -> p b r", r=4)

                    s1 = smp.tile([128, NBLK], F32, tag="s1")
                    nc.vector.reduce_sum(s1[:], g4(e1[:]), axis=X)
                    s2 = smp.tile([128, NBLK], F32, tag="s2")
                    nc.vector.reduce_sum(s2[:], g4(e2[:]), axis=X)
                    ssum = smp.tile([128, NBLK], F32, tag="ssum")
                    nc.vector.tensor_add(ssum[:], s1[:], s2[:])

                    pm = sbp.tile([128, MROWS], F32, tag="pm")
                    nc.vector.tensor_mul(pm[:], e1[:], kv1[:])
                    q1 = smp.tile([128, NBLK], F32, tag="q1")
                    nc.vector.reduce_sum(q1[:], g4(pm[:]), axis=X)
                    pm2 = sbp.tile([128, MROWS], F32, tag="pm2")
                    nc.vector.tensor_mul(pm2[:], e2[:], kv2[:])
                    q2 = smp.tile([128, NBLK], F32, tag="q2")
                    nc.vector.reduce_sum(q2[:], g4(pm2[:]), axis=X)
                    qsum = smp.tile([128, NBLK], F32, tag="qsum")
                    nc.vector.tensor_add(qsum[:], q1[:], q2[:])

                    rinv = smp.tile([128, NBLK], F32, tag="rinv")
                    nc.vector.reciprocal(rinv[:], ssum[:])
                    nc.vector.tensor_mul(pooled[:, j, :], qsum[:], rinv[:])
                    # squared copy for the RMSNorm variance (pre-rope, which
                    # matches the reference: rope is norm-preserving and the
                    # norm scale is applied at the very end)
                    nc.scalar.activation(
                        sq[:, j, :], pooled[:, j, :],
                        mybir.ActivationFunctionType.Square)

                    if j == 3:
                        # rope on chunk 3 (ch 384..511; rows 64.. are rope),
                        # right after its pooling so it overlaps group 0
                        sw_ps = auxp.tile([128, NBLK], F32, tag="swap")
                        nc.tensor.matmul(sw_ps[:], lhsT=psw_sb[:],
                                         rhs=pooled[:, 3, :],
                                         start=True, stop=True)
                        cslice = cos_sb[:, mch * NBLK:(mch + 1) * NBLK]
                        sslice = sin_sb[:, mch * NBLK:(mch + 1) * NBLK]
                        tmpc = smp.tile([128, NBLK], F16, tag="tmpc")
                        nc.vector.tensor_mul(tmpc[:], pooled[:, 3, :], cslice)
                        tmps = smp.tile([128, NBLK], F16, tag="tmps")
                        nc.vector.tensor_mul(tmps[:], sw_ps[:], sslice)
                        nc.vector.tensor_add(pooled[:, 3, :], tmpc[:], tmps[:])

                # ---- RMSNorm stats: var[blk] via accumulating matmuls with
                # lhsT=sq chunks -> PSUM [128(blk), 1]
                var_ps = auxp.tile([128, 1], F32, tag="var")
                for j in range(4):
                    nc.tensor.matmul(var_ps[:], lhsT=sq[:, j, :],
                                     rhs=ones_sb[:, 0:1],
                                     start=(j == 0), stop=(j == 3))
                sd_col = smp.tile([128, 1], F32, tag="sd_col")
                # sd = sqrt(var + 512*eps) = sqrt(512) * sqrt(var/512 + eps);
                # the extra 1/sqrt(512) folds the Hadamard normalization (H
                # rows are +-1 on device)
                nc.scalar.activation(sd_col[:], var_ps[:],
                                     mybir.ActivationFunctionType.Sqrt,
                                     scale=1.0, bias=eps_sb[:, 0:1])
                scale_col = smp.tile([128, 1], F32, tag="scale_col")
                nc.vector.reciprocal(scale_col[:], sd_col[:])

                # ---- Hadamard: out[blk, c'] = sum_c pooled[c, blk] H[c, c']
                # (fp16 operands: 1 cycle/row vs 4 for fp32; H rows are +-1, exact)
                had_ps = hadp.tile([128, 512], F32, tag="had")
                for j in range(4):
                    nc.tensor.matmul(had_ps[:],
                                     lhsT=pooled[:, j, :],
                                     rhs=h_sb[:, j, :],
                                     start=(j == 0), stop=(j == 3))
                out_sb = outp.tile([128, 512], F32, tag="out")
                nc.scalar.activation(out_sb[:], had_ps[:],
                                     mybir.ActivationFunctionType.Copy,
                                     scale=scale_col[:, 0:1])
                nc.gpsimd.dma_start(
                    out=out_d[mch * NBLK:(mch + 1) * NBLK, :], in_=out_sb[:])
    nc.compile()
    return nc


def _prep_shared(W_kv, W_gate, ape, norm_w, H):
    W = np.concatenate([W_kv, W_gate], axis=0).astype(np.float32)  # [2048, 4096]
    Wb = W.astype(BF16)
    wp = np.ascontiguousarray(
        Wb.T.reshape(DCH, 128, 16, 128).transpose(2, 1, 0, 3))  # [16,128,32,128]
    ape_t = np.ascontiguousarray(
        ape.astype(np.float32).T.reshape(8, 128, 4).transpose(1, 0, 2)
    ).reshape(128, 32)
    psw = np.zeros((128, 128), np.float16)
    idx = np.arange(64)
    psw[idx, idx] = 1.0
    k2 = np.arange(0, 64, 2)
    psw[64 + k2 + 1, 64 + k2] = 1.0
    psw[64 + k2, 64 + k2 + 1] = 1.0
    hm = np.ascontiguousarray(
        (norm_w.astype(np.float32)[:, None] * H.astype(np.float32)
         * np.sqrt(512.0, dtype=np.float32))
        .reshape(4, 128, 512).transpose(1, 0, 2)).astype(np.float16)
    return wp, ape_t, psw, hm


def _hadamard(n):
    h = np.array([[1.0]], dtype=np.float32)
    while h.shape[0] < n:
        h = np.block([[h, h], [h, -h]])
    return (h / np.sqrt(n)).astype(np.float32)


def _make_in_maps(x, W_kv, W_gate, ape, norm_w, freqs_cis):
    b, s, _ = x.shape
    H = _hadamard(512)
    wp, ape_t, psw, hm = _prep_shared(W_kv, W_gate, ape, norm_w, H)

    # truncate-to-bf16 (hi-16 planes of the f32 words) and transpose once
    xh = x.reshape(b * s, DIM).view(BF16)[:, 1::2]
    xT = np.ascontiguousarray(xh.T)  # [4096, 16384]
    fr = freqs_cis[:, :, 0]  # [nb, 32]
    fi = freqs_cis[:, :, 1]

    in_maps = []
    for c in range(N_CORES):
        batch, half = c // 2, c % 2
        R0 = batch * s + half * ROWS
        xs = np.zeros((DIM, XS_ROWS), BF16)
        xs[:, 16:] = xT[:, R0:R0 + ROWS]
        if half == 1:
            xs[:, :16] = xT[:, R0 - 16:R0]

        g0 = half * 512
        bi = np.arange(g0, g0 + 512)
        cos_t = np.zeros((128, 512), np.float16)
        cos_t[:64] = 1.0
        cos_t[64:] = np.repeat(fr[bi].T, 2, axis=0).astype(np.float16)
        sin_t = np.zeros((128, 512), np.float16)
        st = np.repeat(fi[bi].T, 2, axis=0)
        st[0::2] *= -1.0
        sin_t[64:] = st.astype(np.float16)

        zmask = np.full((128, 1), 0.0 if half == 0 else 1.0, np.float32)
        in_maps.append({
            "xs": xs, "wp": wp, "ape_t": ape_t,
            "cos_t": cos_t, "sin_t": sin_t, "psw": psw,
            "hmat": hm, "zmask": zmask,
        })
    return in_maps


def kernel(x, W_kv, W_gate, ape, norm_w, freqs_cis, start_pos=0):
    x = np.asarray(x, dtype=np.float32)
    W_kv = np.asarray(W_kv, dtype=np.float32)
    W_gate = np.asarray(W_gate, dtype=np.float32)
    ape = np.asarray(ape, dtype=np.float32)
    norm_w = np.asarray(norm_w, dtype=np.float32)
    freqs_cis = np.asarray(freqs_cis, dtype=np.float32)

    b, s, _ = x.shape
    nb = s // 4
    assert (b, s) == (4, 4096), (b, s)

    if "nc" not in _CACHE:
        _CACHE["nc"] = _build()
    nc = _CACHE["nc"]

    in_maps = _make_in_maps(x, W_kv, W_gate, ape, norm_w, freqs_cis)

    trace = os.environ.get("KERNEL_TRACE", "") not in ("", "0")
    res = run_bass_kernel_spmd(nc, in_maps, core_ids=list(range(N_CORES)),
                               trace=trace)
    kernel.last_results = res
    out = np.concatenate([res.results[c]["out"] for c in range(N_CORES)], axis=0)
    return np.ascontiguousarray(out.reshape(b, nb, 512))


# revision 12
# speedup vs baseline: 1.2525x; 1.0362x over previous
"""Trainium2 Bass kernel for nn_Compressor (sparse_attention block compressor).

Math (reference):
  proj = x @ [W_kv; W_gate]^T            # [b*s, 2048]
  kv   = proj[:, :1024] + ape[s%4]       # blockwise (RATIO=4) abs-pos bias
  sc   = proj[:, 1024:]
  window(blk) = {prev blk rows, ch 0:512} + {cur blk rows, ch 512:1024}
  pooled[blk, c] = softmax-gated channelwise pool over the 8-entry window
  out = (RMSNorm(pooled) -> rope on ch 448:512) @ H  (512x512 Hadamard)

Distribution: 8 cores, data-parallel over (batch, seq-half). Each core owns
2048 seq rows = 512 blocks; the 1-block halo is handled by shifting the
matmul rhs window by 4 rows (the x^T input carries 16 halo rows per m-chunk).

Key implementation tricks:
  * x^T in bf16 obtained host-side by truncating f32 to the hi-16 planes,
    then re-tiled per m-chunk to [128(dpart), 32(dchunk), 528(m)] so each
    DMA descriptor covers a 8.4KB contiguous run (DMA descriptor rate, not
    bandwidth, is the scarce resource for thin transfers).
  * All input DMAs share the sync queue in a hand-chosen order (first weight
    tile split in 4 -> mch0 x^T quarters interleaved with the next weight
    tiles), so the PE ramps at the DMA-feasibility limit and cross-queue
    descriptor round-robin can't starve anything.
  * Projections: W^T tiles stationary (lhsT), x^T moving -> PSUM layout
    [channels(part), m(free)], so the whole softmax pooling is free-axis
    DVE/ACT work and the halo is a free-axis slice offset.
  * Softmax without max-subtraction (scores are ~N(0,1.3); fp32 exp cannot
    overflow; block-0 masking is a 0/1 multiply on exp with a per-core mask).
  * Score passes run before kv passes in each group, so the softmax sums and
    reciprocal are off the critical path.
  * RMSNorm channel reduction via tiny accumulating matmuls with
    lhsT=pooled^2 chunks -> var lands with partition=block, matching the
    Hadamard output layout; scale applied per-partition after
    the final Hadamard matmul (everything in between is linear).
  * Hadamard matmuls in fp16 with H rows +-1 (exact); the 1/sqrt(512)
    normalization folds into the RMSNorm scale. 1 cycle/row vs 4 for fp32.
  * j-groups in order (3,0,1,2): the rope fix-up on chunk 3 overlaps later
    groups; the very last group is split in half-N so its pooling chain
    overlaps the second half's matmuls.
"""

import os
import numpy as np
import ml_dtypes

import concourse.bass as bass
import concourse.bacc as bacc
import concourse.mybir as mybir
from concourse.tile import TileContext
from concourse.bass_utils import run_bass_kernel_spmd

BF16 = ml_dtypes.bfloat16
F32 = mybir.dt.float32
F16 = mybir.dt.float16
BF = mybir.dt.bfloat16

N_CORES = 8
DIM = 4096
ROWS = 2048         # own rows per core
MCH = 4             # m-chunks per core
MROWS = 512         # rows per m-chunk
NBLK = 128          # blocks per m-chunk
DCH = 32            # d chunks of 128
WSUB = 4            # sub-DMAs for the very first weight tile
XQ = 4              # x^T quarter-DMAs per m-chunk (8 d-chunks each)
# o-chunks 0..3 kv-first(prev), 4..7 kv-second(cur), 8..11 sc-first, 12..15 sc-second
FIRST_HALF = (0, 1, 2, 3, 8, 9, 10, 11)

_CACHE = {}


def _build():
    nc = bacc.Bacc("TRN2", target_bir_lowering=False, debug=False,
                   num_devices=N_CORES)
    xs2 = nc.dram_tensor("xs2", [MCH, 128, DCH, 528], BF, kind="ExternalInput")
    wp = nc.dram_tensor("wp", [16, 128, DCH, 128], BF, kind="ExternalInput")
    ape_d = nc.dram_tensor("ape_t", [128, 32], F32, kind="ExternalInput")
    cos_d = nc.dram_tensor("cos_t", [128, 512], F16, kind="ExternalInput")
    sin_d = nc.dram_tensor("sin_t", [128, 512], F16, kind="ExternalInput")
    psw_d = nc.dram_tensor("psw", [128, 128], F16, kind="ExternalInput")
    h_d = nc.dram_tensor("hmat", [128, 4, 512], F16, kind="ExternalInput")
    zmask_d = nc.dram_tensor("zmask", [128, 1], F32, kind="ExternalInput")
    out_d = nc.dram_tensor("out", [4 * NBLK, 512], F32, kind="ExternalOutput")

    X = mybir.AxisListType.X
    AF = mybir.ActivationFunctionType

    with TileContext(nc) as tc:
        with (
            tc.tile_pool(name="const", bufs=1) as constp,
            tc.tile_pool(name="xt", bufs=2) as xtp,
            tc.tile_pool(name="wt", bufs=2) as wtp,
            tc.tile_pool(name="sb", bufs=2) as sbp,
            tc.tile_pool(name="pl", bufs=2) as plp,
            tc.tile_pool(name="sm", bufs=2) as smp,
            tc.tile_pool(name="osb", bufs=2) as outp,
            tc.tile_pool(name="proj", bufs=4, space="PSUM") as projp,
            tc.tile_pool(name="had", bufs=1, space="PSUM") as hadp,
            tc.tile_pool(name="aux", bufs=1, space="PSUM") as auxp,
        ):
            # ---- constants (scalar queue: off the main input queue) ----
            ape_sb = constp.tile([128, 32], F32, tag="ape")
            nc.scalar.dma_start(out=ape_sb[:], in_=ape_d[:, :])
            cos_sb = constp.tile([128, 512], F16, tag="cos")
            nc.scalar.dma_start(out=cos_sb[:], in_=cos_d[:, :])
            sin_sb = constp.tile([128, 512], F16, tag="sin")
            nc.scalar.dma_start(out=sin_sb[:], in_=sin_d[:, :])
            psw_sb = constp.tile([128, 128], F16, tag="psw")
            nc.scalar.dma_start(out=psw_sb[:], in_=psw_d[:, :])
            h_sb = constp.tile([128, 4, 512], F16, tag="h")
            nc.scalar.dma_start(out=h_sb[:], in_=h_d[:, :, :])
            zmask_sb = constp.tile([128, 1], F32, tag="zmask")
            nc.scalar.dma_start(out=zmask_sb[:], in_=zmask_d[:, :])
            ones_sb = constp.tile([128, 1], F32, tag="ones")
            nc.vector.memset(ones_sb[:], 1.0)
            eps_sb = constp.tile([128, 1], F32, tag="eps")
            nc.vector.memset(eps_sb[:], 512e-6)

            def wdma(oc, t):
                w = wtp.tile([128, DCH, 128], BF, tag=f"w{t}")
                nc.sync.dma_start(out=w[:], in_=wp[oc])
                return w

            def xdma(mch, q):
                xt = xtp.tile([128, 8, 528], BF, tag=f"xq{q}")
                nc.sync.dma_start(out=xt[:], in_=xs2[mch, :, 8 * q:8 * (q + 1), :])
                return xt

            # ---- mch0 startup: hand-ordered single-queue DMA interleave so
            # the PE ramp is DMA-feasibility-limited (~1us start, no starve).
            # The first pass is t2 (oc11), so that tile is the split one.
            w0subs = []
            w = constp.tile([128, 8, 128], BF, tag="w0s0")
            nc.sync.dma_start(out=w[:], in_=wp[11, :, 0:8, :])
            w0subs.append(w)
            xq0 = [xdma(0, 0)]
            for s in range(1, WSUB):
                w = constp.tile([128, 8, 128], BF, tag=f"w0s{s}")
                nc.sync.dma_start(out=w[:], in_=wp[11, :, 8 * s:8 * (s + 1), :])
                w0subs.append(w)
            xq0.append(xdma(0, 1))
            wj3 = {2: None, 3: wdma(15, 3)}
            xq0.append(xdma(0, 2))
            xq0.append(xdma(0, 3))
            wj3[0] = wdma(3, 0)
            wj3[1] = wdma(7, 1)

            for mch in range(MCH):
                # x^T quarters: [128(dpart), 8(dchunk), 528(m)] bf16; m slot
                # s <-> own row 512*mch + s - 16; slots 12..15 = halo rows.
                if mch == 0:
                    xqs = xq0
                else:
                    xqs = [xdma(mch, q) for q in range(XQ)]

                pooled = plp.tile([128, 4, NBLK], F16, tag="pooled")
                sq = plp.tile([128, 4, NBLK], F32, tag="sq")

                def emit_group(j, b0, nblk, wtiles, pstiles=None):
                    """Projection + pooling for chunk-group j, blocks
                    [b0, b0+nblk) of this m-chunk. wtiles: {t: tile or None}.
                    For half-N groups, pstiles carries full-width psum tiles
                    shared by both halves (separate column ranges)."""
                    nrows = 4 * nblk
                    m0 = 4 * b0
                    group = {}
                    # score passes (t2,t3) first, kv passes (t0,t1) last: the
                    # softmax sums + reciprocal run during the kv matmuls
                    for t in (2, 3, 0, 1):
                        oc = j + 4 * t
                        if wtiles.get(t) is None:
                            def wslice(d):
                                return w0subs[d // 8][:, d % 8, :]
                        else:
                            def wslice(d, w=wtiles[t]):
                                return w[:, d, :]
                        if pstiles is None:
                            ps_t = projp.tile([128, nrows], F32, tag="proj")
                            ps = ps_t[:]
                        else:
                            ps = pstiles[t][:, m0:m0 + nrows]
                        off = (12 if oc in FIRST_HALF else 16) + m0
                        for d in range(DCH):
                            nc.tensor.matmul(
                                ps,
                                lhsT=wslice(d),
                                rhs=xqs[d // 8][:, d % 8, off:off + nrows],
                                start=(d == 0),
                                stop=(d == DCH - 1),
                                skip_group_check=(pstiles is not None),
                            )
                        if t < 2:
                            # kv chunk: PSUM -> SBUF with ape bias added
                            kv = sbp.tile([128, nrows], F32, tag=f"kv{t}_{nblk}")
                            a = oc  # ape chunk = kv o-chunk (0..7)
                            ape_ap = (ape_sb[:, 4 * a:4 * a + 4]
                                      .unsqueeze(1).to_broadcast((128, nblk, 4)))
                            nc.vector.tensor_add(
                                kv[:].rearrange("p (b r) -> p b r", r=4),
                                ps.rearrange("p (b r) -> p b r", r=4),
                                ape_ap,
                            )
                            group[f"kv{t}"] = kv
                        else:
                            # score chunk: e = exp(psum) straight to SBUF
                            e = sbp.tile([128, nrows], F32, tag=f"e{t}_{nblk}")
                            nc.scalar.activation(e[:], ps, AF.Exp)
                            if mch == 0 and b0 == 0 and t == 2:
                                # block-0 of even cores: zero the 4 prev-window
                                # weights (zmask = 0 even / 1 odd)
                                nc.vector.tensor_scalar_mul(
                                    e[:, 0:4], e[:, 0:4], zmask_sb[:, 0:1])
                            group[f"e{t}"] = e

                        if t == 3:
                            # softmax denominator, as soon as both e's exist
                            e1, e2 = group["e2"], group["e3"]
                            s1 = smp.tile([128, nblk], F32, tag=f"s1_{nblk}")
                            nc.vector.reduce_sum(
                                s1[:], e1[:].rearrange("p (b r) -> p b r", r=4),
                                axis=X)
                            s2 = smp.tile([128, nblk], F32, tag=f"s2_{nblk}")
                            nc.vector.reduce_sum(
                                s2[:], e2[:].rearrange("p (b r) -> p b r", r=4),
                                axis=X)
                            ssum = smp.tile([128, nblk], F32, tag=f"ss_{nblk}")
                            nc.vector.tensor_add(ssum[:], s1[:], s2[:])
                            rinv = smp.tile([128, nblk], F32, tag=f"ri_{nblk}")
                            nc.vector.reciprocal(rinv[:], ssum[:])
                            group["rinv"] = rinv
                        if t == 0:
                            # first kv ready: weighted sum of the prev-window
                            pm = sbp.tile([128, nrows], F32, tag=f"pm_{nblk}")
                            nc.vector.tensor_mul(pm[:], group["e2"][:],
                                                 group["kv0"][:])
                            q1 = smp.tile([128, nblk], F32, tag=f"q1_{nblk}")
                            nc.vector.reduce_sum(
                                q1[:], pm[:].rearrange("p (b r) -> p b r", r=4),
                                axis=X)
                            group["q1"] = q1

                    pm2 = sbp.tile([128, nrows], F32, tag=f"pm2_{nblk}")
                    nc.vector.tensor_mul(pm2[:], group["e3"][:], group["kv1"][:])
                    q2 = smp.tile([128, nblk], F32, tag=f"q2_{nblk}")
                    nc.vector.reduce_sum(
                        q2[:], pm2[:].rearrange("p (b r) -> p b r", r=4), axis=X)
                    qsum = smp.tile([128, nblk], F32, tag=f"qs_{nblk}")
                    nc.vector.tensor_add(qsum[:], group["q1"][:], q2[:])
                    nc.vector.tensor_mul(pooled[:, j, b0:b0 + nblk], qsum[:],
                                         group["rinv"][:])
                    # squared copy for the RMSNorm variance (pre-rope: rope is
                    # norm-preserving; the norm scale is applied at the end)
                    nc.scalar.activation(sq[:, j, b0:b0 + nblk],
                                         pooled[:, j, b0:b0 + nblk], AF.Square)

                # j order: chunk 3 first so its rope fix-up overlaps later
                # groups; last group of the last m-chunk split in half-N so
                # its pooling chain overlaps the second half's matmuls.
                for j in (3, 0, 1, 2):
                    if mch == 0 and j == 3:
                        wtiles = wj3
                    else:
                        wtiles = {t: wdma(j + 4 * t, t) for t in (2, 3, 0, 1)}
                    if mch == MCH - 1 and j == 2:
                        pst = {}
                        for t in (2, 3, 0, 1):
                            ps_full = projp.tile([128, MROWS], F32, tag="proj")
                            pst[t] = ps_full
                        emit_group(j, 0, NBLK // 2, wtiles, pst)
                        emit_group(j, NBLK // 2, NBLK // 2, wtiles, pst)
                    else:
                        emit_group(j, 0, NBLK, wtiles)

                    if j == 3:
                        # rope on chunk 3 (ch 384..511; rows 64.. are rope),
                        # right after its pooling so it overlaps group 0
                        sw_ps = auxp.tile([128, NBLK], F32, tag="swap")
                        nc.tensor.matmul(sw_ps[:], lhsT=psw_sb[:],
                                         rhs=pooled[:, 3, :],
                                         start=True, stop=True)
                        cslice = cos_sb[:, mch * NBLK:(mch + 1) * NBLK]
                        sslice = sin_sb[:, mch * NBLK:(mch + 1) * NBLK]
                        tmpc = smp.tile([128, NBLK], F16, tag="tmpc")
                        nc.vector.tensor_mul(tmpc[:], pooled[:, 3, :], cslice)
                        tmps = smp.tile([128, NBLK], F16, tag="tmps")
                        nc.vector.tensor_mul(tmps[:], sw_ps[:], sslice)
                        nc.vector.tensor_add(pooled[:, 3, :], tmpc[:], tmps[:])

                # ---- RMSNorm stats: var[blk] via accumulating matmuls with
                # lhsT=sq chunks -> PSUM [128(blk), 1]; then scale = Rsqrt
                var_ps = auxp.tile([128, 1], F32, tag="var")
                for jj in range(4):
                    nc.tensor.matmul(var_ps[:], lhsT=sq[:, jj, :],
                                     rhs=ones_sb[:, 0:1],
                                     start=(jj == 0), stop=(jj == 3))
                # scale = 1/sqrt(var + 512*eps) = (1/sqrt(512)) / sqrt(
                # var/512 + eps); the 1/sqrt(512) folds the Hadamard
                # normalization (H rows are +-1 on device)
                sd_col = smp.tile([128, 1], F32, tag="sd_col")
                nc.scalar.activation(sd_col[:], var_ps[:], AF.Sqrt,
                                     scale=1.0, bias=eps_sb[:, 0:1])
                scale_col = smp.tile([128, 1], F32, tag="scale_col")
                nc.vector.reciprocal(scale_col[:], sd_col[:])

                # ---- Hadamard: out[blk, c'] = sum_c pooled[c, blk] H[c, c']
                # (fp16 operands: 1 cycle/row vs 4 for fp32; H rows +-1 exact)
                had_ps = hadp.tile([128, 512], F32, tag="had")
                for jj in range(4):
                    nc.tensor.matmul(had_ps[:],
                                     lhsT=pooled[:, jj, :],
                                     rhs=h_sb[:, jj, :],
                                     start=(jj == 0), stop=(jj == 3))
                out_sb = outp.tile([128, 512], F32, tag="out")
                nc.scalar.activation(out_sb[:], had_ps[:], AF.Copy,
                                     scale=scale_col[:, 0:1])
                nc.gpsimd.dma_start(
                    out=out_d[mch * NBLK:(mch + 1) * NBLK, :], in_=out_sb[:])
    nc.compile()
    return nc


def _prep_shared(W_kv, W_gate, ape, norm_w, H):
    W = np.concatenate([W_kv, W_gate], axis=0).astype(np.float32)  # [2048, 4096]
    Wb = W.astype(BF16)
    wp = np.ascontiguousarray(
        Wb.T.reshape(DCH, 128, 16, 128).transpose(2, 1, 0, 3))  # [16,128,32,128]
    ape_t = np.ascontiguousarray(
        ape.astype(np.float32).T.reshape(8, 128, 4).transpose(1, 0, 2)
    ).reshape(128, 32)
    psw = np.zeros((128, 128), np.float16)
    idx = np.arange(64)
    psw[idx, idx] = 1.0
    k2 = np.arange(0, 64, 2)
    psw[64 + k2 + 1, 64 + k2] = 1.0
    psw[64 + k2, 64 + k2 + 1] = 1.0
    hm = np.ascontiguousarray(
        (norm_w.astype(np.float32)[:, None] * H.astype(np.float32)
         * np.sqrt(512.0, dtype=np.float32))
        .reshape(4, 128, 512).transpose(1, 0, 2)).astype(np.float16)
    return wp, ape_t, psw, hm


def _hadamard(n):
    h = np.array([[1.0]], dtype=np.float32)
    while h.shape[0] < n:
        h = np.block([[h, h], [h, -h]])
    return (h / np.sqrt(n)).astype(np.float32)


def _make_in_maps(x, W_kv, W_gate, ape, norm_w, freqs_cis):
    b, s, _ = x.shape
    H = _hadamard(512)
    wp, ape_t, psw, hm = _prep_shared(W_kv, W_gate, ape, norm_w, H)

    # truncate-to-bf16 (hi-16 planes of the f32 words) and transpose once
    xh = x.reshape(b * s, DIM).view(BF16)[:, 1::2]
    xT = np.ascontiguousarray(xh.T)  # [4096, 16384]
    fr = freqs_cis[:, :, 0]  # [nb, 32]
    fi = freqs_cis[:, :, 1]

    in_maps = []
    for c in range(N_CORES):
        batch, half = c // 2, c % 2
        R0 = batch * s + half * ROWS
        xs = np.zeros((DIM, 16 + ROWS), BF16)
        xs[:, 16:] = xT[:, R0:R0 + ROWS]
        if half == 1:
            xs[:, :16] = xT[:, R0 - 16:R0]
        # per-m-chunk windows, re-tiled so every DMA descriptor is a
        # contiguous 8.4KB run: [mch][128 dpart][32 dchunk][528 m]
        xs2 = np.empty((MCH, 128, DCH, 528), BF16)
        for m in range(MCH):
            win = xs[:, 512 * m:512 * m + 528]
            xs2[m] = win.reshape(DCH, 128, 528).transpose(1, 0, 2)

        g0 = half * 512
        bi = np.arange(g0, g0 + 512)
        cos_t = np.zeros((128, 512), np.float16)
        cos_t[:64] = 1.0
        cos_t[64:] = np.repeat(fr[bi].T, 2, axis=0).astype(np.float16)
        sin_t = np.zeros((128, 512), np.float16)
        st = np.repeat(fi[bi].T, 2, axis=0)
        st[0::2] *= -1.0
        sin_t[64:] = st.astype(np.float16)

        zmask = np.full((128, 1), 0.0 if half == 0 else 1.0, np.float32)
        in_maps.append({
            "xs2": xs2, "wp": wp, "ape_t": ape_t,
            "cos_t": cos_t, "sin_t": sin_t, "psw": psw,
            "hmat": hm, "zmask": zmask,
        })
    return in_maps


def kernel(x, W_kv, W_gate, ape, norm_w, freqs_cis, start_pos=0):
    x = np.asarray(x, dtype=np.float32)
    W_kv = np.asarray(W_kv, dtype=np.float32)
    W_gate = np.asarray(W_gate, dtype=np.float32)
    ape = np.asarray(ape, dtype=np.float32)
    norm_w = np.asarray(norm_w, dtype=np.float32)
    freqs_cis = np.asarray(freqs_cis, dtype=np.float32)

    b, s, _ = x.shape
    nb = s // 4
    assert (b, s) == (4, 4096), (b, s)

    if "nc" not in _CACHE:
        _CACHE["nc"] = _build()
    nc = _CACHE["nc"]

    in_maps = _make_in_maps(x, W_kv, W_gate, ape, norm_w, freqs_cis)

    trace = os.environ.get("KERNEL_TRACE", "") not in ("", "0")
    res = run_bass_kernel_spmd(nc, in_maps, core_ids=list(range(N_CORES)),
                               trace=trace)
    kernel.last_results = res
    out = np.concatenate([res.results[c]["out"] for c in range(N_CORES)], axis=0)
    return np.ascontiguousarray(out.reshape(b, nb, 512))


# revision 15
# speedup vs baseline: 1.2551x; 1.0020x over previous
"""Trainium2 Bass kernel for nn_Compressor (sparse_attention block compressor).

Math (reference):
  proj = x @ [W_kv; W_gate]^T            # [b*s, 2048]
  kv   = proj[:, :1024] + ape[s%4]       # blockwise (RATIO=4) abs-pos bias
  sc   = proj[:, 1024:]
  window(blk) = {prev blk rows, ch 0:512} + {cur blk rows, ch 512:1024}
  pooled[blk, c] = softmax-gated channelwise pool over the 8-entry window
  out = (RMSNorm(pooled) -> rope on ch 448:512) @ H  (512x512 Hadamard)

Distribution: 8 cores, data-parallel over (batch, seq-half). Each core owns
2048 seq rows = 512 blocks; the 1-block halo is handled by shifting the
matmul rhs window by 4 rows (the x^T input carries 16 halo rows per m-chunk).

Key implementation tricks:
  * x^T in bf16 obtained host-side by truncating f32 to the hi-16 planes,
    then re-tiled per m-chunk to [128(dpart), 32(dchunk), 528(m)] so each
    DMA descriptor covers a 8.4KB contiguous run (DMA descriptor rate, not
    bandwidth, is the scarce resource for thin transfers).
  * All input DMAs share the sync queue in a hand-chosen order (first weight
    tile split in 4 -> mch0 x^T quarters interleaved with the next weight
    tiles), so the PE ramps at the DMA-feasibility limit and cross-queue
    descriptor round-robin can't starve anything.
  * Projections: W^T tiles stationary (lhsT), x^T moving -> PSUM layout
    [channels(part), m(free)], so the whole softmax pooling is free-axis
    DVE/ACT work and the halo is a free-axis slice offset.
  * Softmax without max-subtraction (scores are ~N(0,1.3); fp32 exp cannot
    overflow; block-0 masking is a 0/1 multiply on exp with a per-core mask).
  * Score passes run before kv passes in each group, so the softmax sums and
    reciprocal are off the critical path.
  * RMSNorm channel reduction via tiny accumulating matmuls with
    lhsT=pooled^2 chunks -> var lands with partition=block, matching the
    Hadamard output layout; scale applied per-partition after
    the final Hadamard matmul (everything in between is linear).
  * Hadamard matmuls in fp16 with H rows +-1 (exact); the 1/sqrt(512)
    normalization folds into the RMSNorm scale. 1 cycle/row vs 4 for fp32.
  * j-groups in order (3,0,1,2): the rope fix-up on chunk 3 overlaps later
    groups; the very last group is split in half-N so its pooling chain
    overlaps the second half's matmuls.
"""

import os
import numpy as np
import ml_dtypes

import concourse.bass as bass
import concourse.bacc as bacc
import concourse.mybir as mybir
from concourse.tile import TileContext
from concourse.bass_utils import run_bass_kernel_spmd

BF16 = ml_dtypes.bfloat16
F32 = mybir.dt.float32
F16 = mybir.dt.float16
BF = mybir.dt.bfloat16

N_CORES = 8
DIM = 4096
ROWS = 2048         # own rows per core
MCH = 4             # m-chunks per core
MROWS = 512         # rows per m-chunk
NBLK = 128          # blocks per m-chunk
DCH = 32            # d chunks of 128
WSUB = 4            # sub-DMAs for the very first weight tile
XQ = 4              # x^T quarter-DMAs per m-chunk (8 d-chunks each)
# o-chunks 0..3 kv-first(prev), 4..7 kv-second(cur), 8..11 sc-first, 12..15 sc-second
FIRST_HALF = (0, 1, 2, 3, 8, 9, 10, 11)

_CACHE = {}


def _build():
    nc = bacc.Bacc("TRN2", target_bir_lowering=False, debug=False,
                   num_devices=N_CORES)
    xs2 = nc.dram_tensor("xs2", [MCH, 128, DCH, 528], BF, kind="ExternalInput")
    wp = nc.dram_tensor("wp", [16, 128, DCH, 128], BF, kind="ExternalInput")
    ape_d = nc.dram_tensor("ape_t", [128, 32], F32, kind="ExternalInput")
    cos_d = nc.dram_tensor("cos_t", [128, 512], F16, kind="ExternalInput")
    sin_d = nc.dram_tensor("sin_t", [128, 512], F16, kind="ExternalInput")
    psw_d = nc.dram_tensor("psw", [128, 128], F16, kind="ExternalInput")
    h_d = nc.dram_tensor("hmat", [128, 4, 512], F16, kind="ExternalInput")
    zmask_d = nc.dram_tensor("zmask", [128, 1], F32, kind="ExternalInput")
    out_d = nc.dram_tensor("out", [4 * NBLK, 512], F32, kind="ExternalOutput")

    X = mybir.AxisListType.X
    AF = mybir.ActivationFunctionType

    with TileContext(nc) as tc:
        with (
            tc.tile_pool(name="const", bufs=1) as constp,
            tc.tile_pool(name="xt", bufs=2) as xtp,
            tc.tile_pool(name="wt", bufs=2) as wtp,
            tc.tile_pool(name="sb", bufs=2) as sbp,
            tc.tile_pool(name="pl", bufs=2) as plp,
            tc.tile_pool(name="sm", bufs=2) as smp,
            tc.tile_pool(name="osb", bufs=1) as outp,
            tc.tile_pool(name="proj", bufs=4, space="PSUM") as projp,
            tc.tile_pool(name="had", bufs=1, space="PSUM") as hadp,
            tc.tile_pool(name="aux", bufs=1, space="PSUM") as auxp,
        ):
            # ---- constants (scalar queue: off the main input queue) ----
            ape_sb = constp.tile([128, 32], F32, tag="ape")
            nc.scalar.dma_start(out=ape_sb[:], in_=ape_d[:, :])
            cos_sb = constp.tile([128, 512], F16, tag="cos")
            nc.scalar.dma_start(out=cos_sb[:], in_=cos_d[:, :])
            sin_sb = constp.tile([128, 512], F16, tag="sin")
            nc.scalar.dma_start(out=sin_sb[:], in_=sin_d[:, :])
            psw_sb = constp.tile([128, 128], F16, tag="psw")
            nc.scalar.dma_start(out=psw_sb[:], in_=psw_d[:, :])
            h_sb = constp.tile([128, 4, 512], F16, tag="h")
            nc.scalar.dma_start(out=h_sb[:], in_=h_d[:, :, :])
            zmask_sb = constp.tile([128, 1], F32, tag="zmask")
            nc.scalar.dma_start(out=zmask_sb[:], in_=zmask_d[:, :])
            ones_sb = constp.tile([128, 1], F16, tag="ones")
            nc.vector.memset(ones_sb[:], 1.0)
            eps_sb = constp.tile([128, 1], F32, tag="eps")
            nc.vector.memset(eps_sb[:], 512e-6)

            def wdma(oc, t):
                w = wtp.tile([128, DCH, 128], BF, tag=f"w{t}")
                nc.sync.dma_start(out=w[:], in_=wp[oc])
                return w

            def xdma(mch, q):
                xt = xtp.tile([128, 8, 528], BF, tag=f"xq{q}")
                nc.sync.dma_start(out=xt[:], in_=xs2[mch, :, 8 * q:8 * (q + 1), :])
                return xt

            # ---- mch0 startup: hand-ordered single-queue DMA interleave so
            # the PE ramp is DMA-feasibility-limited. The first two passes
            # (t2=oc11, t3=oc15) run PAIRED per d-chunk, so their combined
            # consumption matches the x^T delivery rate; both weight tiles
            # are split in 4 at the head of the queue.
            w0subs, w1subs = [], []
            w = constp.tile([128, 8, 128], BF, tag="w0s0")
            nc.sync.dma_start(out=w[:], in_=wp[11, :, 0:8, :])
            w0subs.append(w)
            w = constp.tile([128, 8, 128], BF, tag="w1s0")
            nc.sync.dma_start(out=w[:], in_=wp[15, :, 0:8, :])
            w1subs.append(w)
            xq0 = [xdma(0, 0)]
            for s in range(1, WSUB):
                w = constp.tile([128, 8, 128], BF, tag=f"w0s{s}")
                nc.sync.dma_start(out=w[:], in_=wp[11, :, 8 * s:8 * (s + 1), :])
                w0subs.append(w)
                w = constp.tile([128, 8, 128], BF, tag=f"w1s{s}")
                nc.sync.dma_start(out=w[:], in_=wp[15, :, 8 * s:8 * (s + 1), :])
                w1subs.append(w)
            xq0.append(xdma(0, 1))
            xq0.append(xdma(0, 2))
            xq0.append(xdma(0, 3))
            wj3 = {2: None, 3: None}
            wj3[0] = wdma(3, 0)
            wj3[1] = wdma(7, 1)

            for mch in range(MCH):
                # x^T quarters: [128(dpart), 8(dchunk), 528(m)] bf16; m slot
                # s <-> own row 512*mch + s - 16; slots 12..15 = halo rows.
                if mch == 0:
                    xqs = xq0
                else:
                    xqs = [xdma(mch, q) for q in range(XQ)]

                pooled = plp.tile([128, 4, NBLK], F16, tag="pooled")
                sq = plp.tile([128, 4, NBLK], F16, tag="sq")

                def emit_group(j, b0, nblk, wtiles, pstiles=None):
                    """Projection + pooling for chunk-group j, blocks
                    [b0, b0+nblk) of this m-chunk. wtiles: {t: tile or None}.
                    For half-N groups, pstiles carries full-width psum tiles
                    shared by both halves (separate column ranges)."""
                    nrows = 4 * nblk
                    m0 = 4 * b0
                    group = {}
                    paired = wtiles.get(2) is None
                    done_mm = set()
                    # score passes (t2,t3) first, kv passes (t0,t1) last: the
                    # softmax sums + reciprocal run during the kv matmuls
                    for t in (2, 3, 0, 1):
                        oc = j + 4 * t
                        if wtiles.get(t) is None:
                            subs = w0subs if t == 2 else w1subs

                            def wslice(d, subs=subs):
                                return subs[d // 8][:, d % 8, :]
                        else:
                            def wslice(d, w=wtiles[t]):
                                return w[:, d, :]
                        if pstiles is None:
                            ps_t = projp.tile([128, nrows], F32, tag="proj")
                            ps = ps_t[:]
                        else:
                            ps = pstiles[t][:, m0:m0 + nrows]
                        off = (12 if oc in FIRST_HALF else 16) + m0
                        if t not in done_mm:
                            if paired and t == 2:
                                # interleave t2/t3 per d-chunk: combined
                                # consumption rate matches the x^T delivery
                                ps3_t = projp.tile([128, nrows], F32,
                                                   tag="proj")
                                ps3 = ps3_t[:]
                                off3 = (12 if (j + 12) in FIRST_HALF
                                        else 16) + m0
                                for d in range(DCH):
                                    rhs = xqs[d // 8][:, d % 8, :]
                                    nc.tensor.matmul(
                                        ps, lhsT=wslice(d),
                                        rhs=rhs[:, off:off + nrows],
                                        start=(d == 0), stop=(d == DCH - 1),
                                        skip_group_check=True)
                                    nc.tensor.matmul(
                                        ps3,
                                        lhsT=w1subs[d // 8][:, d % 8, :],
                                        rhs=rhs[:, off3:off3 + nrows],
                                        start=(d == 0), stop=(d == DCH - 1),
                                        skip_group_check=True)
                                group["ps3"] = ps3
                                done_mm.add(3)
                            else:
                                for d in range(DCH):
                                    nc.tensor.matmul(
                                        ps,
                                        lhsT=wslice(d),
                                        rhs=xqs[d // 8][:, d % 8,
                                                        off:off + nrows],
                                        start=(d == 0),
                                        stop=(d == DCH - 1),
                                        skip_group_check=(pstiles is not None),
                                    )
                        else:
                            ps = group.pop("ps3")
                        if t < 2:
                            # kv chunk: PSUM -> SBUF with ape bias added
                            kv = sbp.tile([128, nrows], F32, tag=f"kv{t}_{nblk}")
                            a = oc  # ape chunk = kv o-chunk (0..7)
                            ape_ap = (ape_sb[:, 4 * a:4 * a + 4]
                                      .unsqueeze(1).to_broadcast((128, nblk, 4)))
                            nc.vector.tensor_add(
                                kv[:].rearrange("p (b r) -> p b r", r=4),
                                ps.rearrange("p (b r) -> p b r", r=4),
                                ape_ap,
                            )
                            group[f"kv{t}"] = kv
                        else:
                            # score chunk: e = exp(psum) straight to SBUF
                            e = sbp.tile([128, nrows], F32, tag=f"e{t}_{nblk}")
                            nc.scalar.activation(e[:], ps, AF.Exp)
                            if mch == 0 and b0 == 0 and t == 2:
                                # block-0 of even cores: zero the 4 prev-window
                                # weights (zmask = 0 even / 1 odd)
                                nc.vector.tensor_scalar_mul(
                                    e[:, 0:4], e[:, 0:4], zmask_sb[:, 0:1])
                            group[f"e{t}"] = e

                        if t == 3:
                            # softmax denominator, as soon as both e's exist
                            e1, e2 = group["e2"], group["e3"]
                            s1 = smp.tile([128, nblk], F32, tag=f"s1_{nblk}")
                            nc.vector.reduce_sum(
                                s1[:], e1[:].rearrange("p (b r) -> p b r", r=4),
                                axis=X)
                            s2 = smp.tile([128, nblk], F32, tag=f"s2_{nblk}")
                            nc.vector.reduce_sum(
                                s2[:], e2[:].rearrange("p (b r) -> p b r", r=4),
                                axis=X)
                            ssum = smp.tile([128, nblk], F32, tag=f"ss_{nblk}")
                            nc.vector.tensor_add(ssum[:], s1[:], s2[:])
                            rinv = smp.tile([128, nblk], F32, tag=f"ri_{nblk}")
                            nc.vector.reciprocal(rinv[:], ssum[:])
                            group["rinv"] = rinv
                        if t == 0:
                            # first kv ready: weighted sum of the prev-window
                            pm = sbp.tile([128, nrows], F32, tag=f"pm_{nblk}")
                            nc.vector.tensor_mul(pm[:], group["e2"][:],
                                                 group["kv0"][:])
                            q1 = smp.tile([128, nblk], F32, tag=f"q1_{nblk}")
                            nc.vector.reduce_sum(
                                q1[:], pm[:].rearrange("p (b r) -> p b r", r=4),
                                axis=X)
                            group["q1"] = q1

                    pm2 = sbp.tile([128, nrows], F32, tag=f"pm2_{nblk}")
                    nc.vector.tensor_mul(pm2[:], group["e3"][:], group["kv1"][:])
                    q2 = smp.tile([128, nblk], F32, tag=f"q2_{nblk}")
                    nc.vector.reduce_sum(
                        q2[:], pm2[:].rearrange("p (b r) -> p b r", r=4), axis=X)
                    qsum = smp.tile([128, nblk], F32, tag=f"qs_{nblk}")
                    nc.vector.tensor_add(qsum[:], group["q1"][:], q2[:])
                    nc.vector.tensor_mul(pooled[:, j, b0:b0 + nblk], qsum[:],
                                         group["rinv"][:])
                    # squared copy for the RMSNorm variance (pre-rope: rope is
                    # norm-preserving; the norm scale is applied at the end)
                    nc.scalar.activation(sq[:, j, b0:b0 + nblk],
                                         pooled[:, j, b0:b0 + nblk], AF.Square)

                # j order: chunk 3 first so its rope fix-up overlaps later
                # groups; last group of the last m-chunk split in half-N so
                # its pooling chain overlaps the second half's matmuls.
                for j in (3, 0, 1, 2):
                    if mch == 0 and j == 3:
                        wtiles = wj3
                    else:
                        wtiles = {t: wdma(j + 4 * t, t) for t in (2, 3, 0, 1)}
                    if mch == MCH - 1 and j == 2:
                        pst = {}
                        for t in (2, 3, 0, 1):
                            ps_full = projp.tile([128, MROWS], F32, tag="proj")
                            pst[t] = ps_full
                        emit_group(j, 0, NBLK // 2, wtiles, pst)
                        emit_group(j, NBLK // 2, NBLK // 2, wtiles, pst)
                    else:
                        emit_group(j, 0, NBLK, wtiles)

                    if j == 3:
                        # rope on chunk 3 (ch 384..511; rows 64.. are rope),
                        # right after its pooling so it overlaps group 0
                        sw_ps = auxp.tile([128, NBLK], F32, tag="swap")
                        nc.tensor.matmul(sw_ps[:], lhsT=psw_sb[:],
                                         rhs=pooled[:, 3, :],
                                         start=True, stop=True)
                        cslice = cos_sb[:, mch * NBLK:(mch + 1) * NBLK]
                        sslice = sin_sb[:, mch * NBLK:(mch + 1) * NBLK]
                        tmpc = smp.tile([128, NBLK], F16, tag="tmpc")
                        nc.vector.tensor_mul(tmpc[:], pooled[:, 3, :], cslice)
                        tmps = smp.tile([128, NBLK], F16, tag="tmps")
                        nc.vector.tensor_mul(tmps[:], sw_ps[:], sslice)
                        nc.vector.tensor_add(pooled[:, 3, :], tmpc[:], tmps[:])

                # ---- RMSNorm stats: var[blk] via accumulating matmuls with
                # lhsT=sq chunks -> PSUM [128(blk), 1]; then scale = Rsqrt
                var_ps = auxp.tile([128, 1], F32, tag="var")
                for jj in range(4):
                    nc.tensor.matmul(var_ps[:], lhsT=sq[:, jj, :],
                                     rhs=ones_sb[:, 0:1],
                                     start=(jj == 0), stop=(jj == 3))
                # scale = 1/sqrt(var + 512*eps) = (1/sqrt(512)) / sqrt(
                # var/512 + eps); the 1/sqrt(512) folds the Hadamard
                # normalization (H rows are +-1 on device)
                sd_col = smp.tile([128, 1], F32, tag="sd_col")
                nc.scalar.activation(sd_col[:], var_ps[:], AF.Sqrt,
                                     scale=1.0, bias=eps_sb[:, 0:1])
                scale_col = smp.tile([128, 1], F32, tag="scale_col")
                nc.vector.reciprocal(scale_col[:], sd_col[:])

                # ---- Hadamard: out[blk, c'] = sum_c pooled[c, blk] H[c, c']
                # (fp16 operands: 1 cycle/row vs 4 for fp32; H rows +-1 exact)
                had_ps = hadp.tile([128, 512], F32, tag="had")
                for jj in range(4):
                    nc.tensor.matmul(had_ps[:],
                                     lhsT=pooled[:, jj, :],
                                     rhs=h_sb[:, jj, :],
                                     start=(jj == 0), stop=(jj == 3))
                out_sb = outp.tile([128, 512], F32, tag="out")
                nc.scalar.activation(out_sb[:], had_ps[:], AF.Copy,
                                     scale=scale_col[:, 0:1])
                nc.gpsimd.dma_start(
                    out=out_d[mch * NBLK:(mch + 1) * NBLK, :], in_=out_sb[:])
    nc.compile()
    return nc


def _prep_shared(W_kv, W_gate, ape, norm_w, H):
    W = np.concatenate([W_kv, W_gate], axis=0).astype(np.float32)  # [2048, 4096]
    Wb = W.astype(BF16)
    wp = np.ascontiguousarray(
        Wb.T.reshape(DCH, 128, 16, 128).transpose(2, 1, 0, 3))  # [16,128,32,128]
    ape_t = np.ascontiguousarray(
        ape.astype(np.float32).T.reshape(8, 128, 4).transpose(1, 0, 2)
    ).reshape(128, 32)
    psw = np.zeros((128, 128), np.float16)
    idx = np.arange(64)
    psw[idx, idx] = 1.0
    k2 = np.arange(0, 64, 2)
    psw[64 + k2 + 1, 64 + k2] = 1.0
    psw[64 + k2, 64 + k2 + 1] = 1.0
    hm = np.ascontiguousarray(
        (norm_w.astype(np.float32)[:, None] * H.astype(np.float32)
         * np.sqrt(512.0, dtype=np.float32))
        .reshape(4, 128, 512).transpose(1, 0, 2)).astype(np.float16)
    return wp, ape_t, psw, hm


def _hadamard(n):
    h = np.array([[1.0]], dtype=np.float32)
    while h.shape[0] < n:
        h = np.block([[h, h], [h, -h]])
    return (h / np.sqrt(n)).astype(np.float32)


def _make_in_maps(x, W_kv, W_gate, ape, norm_w, freqs_cis):
    b, s, _ = x.shape
    H = _hadamard(512)
    wp, ape_t, psw, hm = _prep_shared(W_kv, W_gate, ape, norm_w, H)

    # truncate-to-bf16 (hi-16 planes of the f32 words) and transpose once
    xh = x.reshape(b * s, DIM).view(BF16)[:, 1::2]
    xT = np.ascontiguousarray(xh.T)  # [4096, 16384]
    fr = freqs_cis[:, :, 0]  # [nb, 32]
    fi = freqs_cis[:, :, 1]

    in_maps = []
    for c in range(N_CORES):
        batch, half = c // 2, c % 2
        R0 = batch * s + half * ROWS
        xs = np.zeros((DIM, 16 + ROWS), BF16)
        xs[:, 16:] = xT[:, R0:R0 + ROWS]
        if half == 1:
            xs[:, :16] = xT[:, R0 - 16:R0]
        # per-m-chunk windows, re-tiled so every DMA descriptor is a
        # contiguous 8.4KB run: [mch][128 dpart][32 dchunk][528 m]
        xs2 = np.empty((MCH, 128, DCH, 528), BF16)
        for m in range(MCH):
            win = xs[:, 512 * m:512 * m + 528]
            xs2[m] = win.reshape(DCH, 128, 528).transpose(1, 0, 2)

        g0 = half * 512
        bi = np.arange(g0, g0 + 512)
        cos_t = np.zeros((128, 512), np.float16)
        cos_t[:64] = 1.0
        cos_t[64:] = np.repeat(fr[bi].T, 2, axis=0).astype(np.float16)
        sin_t = np.zeros((128, 512), np.float16)
        st = np.repeat(fi[bi].T, 2, axis=0)
        st[0::2] *= -1.0
        sin_t[64:] = st.astype(np.float16)

        zmask = np.full((128, 1), 0.0 if half == 0 else 1.0, np.float32)
        in_maps.append({
            "xs2": xs2, "wp": wp, "ape_t": ape_t,
            "cos_t": cos_t, "sin_t": sin_t, "psw": psw,
            "hmat": hm, "zmask": zmask,
        })
    return in_maps


def kernel(x, W_kv, W_gate, ape, norm_w, freqs_cis, start_pos=0):
    x = np.asarray(x, dtype=np.float32)
    W_kv = np.asarray(W_kv, dtype=np.float32)
    W_gate = np.asarray(W_gate, dtype=np.float32)
    ape = np.asarray(ape, dtype=np.float32)
    norm_w = np.asarray(norm_w, dtype=np.float32)
    freqs_cis = np.asarray(freqs_cis, dtype=np.float32)

    b, s, _ = x.shape
    nb = s // 4
    assert (b, s) == (4, 4096), (b, s)

    if "nc" not in _CACHE:
        _CACHE["nc"] = _build()
    nc = _CACHE["nc"]

    in_maps = _make_in_maps(x, W_kv, W_gate, ape, norm_w, freqs_cis)

    trace = os.environ.get("KERNEL_TRACE", "") not in ("", "0")
    res = run_bass_kernel_spmd(nc, in_maps, core_ids=list(range(N_CORES)),
                               trace=trace)
    kernel.last_results = res
    out = np.concatenate([res.results[c]["out"] for c in range(N_CORES)], axis=0)
    return np.ascontiguousarray(out.reshape(b, nb, 512))


# revision 16
# speedup vs baseline: 1.2615x; 1.0051x over previous
"""Trainium2 Bass kernel for nn_Compressor (sparse_attention block compressor).

Math (reference):
  proj = x @ [W_kv; W_gate]^T            # [b*s, 2048]
  kv   = proj[:, :1024] + ape[s%4]       # blockwise (RATIO=4) abs-pos bias
  sc   = proj[:, 1024:]
  window(blk) = {prev blk rows, ch 0:512} + {cur blk rows, ch 512:1024}
  pooled[blk, c] = softmax-gated channelwise pool over the 8-entry window
  out = (RMSNorm(pooled) -> rope on ch 448:512) @ H  (512x512 Hadamard)

Distribution: 8 cores, data-parallel over (batch, seq-half). Each core owns
2048 seq rows = 512 blocks; the 1-block halo is handled by shifting the
matmul rhs window by 4 rows (the x^T input carries 16 halo rows per m-chunk).

Key implementation tricks:
  * x^T in bf16 obtained host-side by truncating f32 to the hi-16 planes,
    then re-tiled per m-chunk to [128(dpart), 32(dchunk), 528(m)] so each
    DMA descriptor covers a 8.4KB contiguous run (DMA descriptor rate, not
    bandwidth, is the scarce resource for thin transfers).
  * All input DMAs share the sync queue in a hand-chosen order (first weight
    tile split in 4 -> mch0 x^T quarters interleaved with the next weight
    tiles), so the PE ramps at the DMA-feasibility limit and cross-queue
    descriptor round-robin can't starve anything.
  * Projections: W^T tiles stationary (lhsT), x^T moving -> PSUM layout
    [channels(part), m(free)], so the whole softmax pooling is free-axis
    DVE/ACT work and the halo is a free-axis slice offset.
  * Softmax without max-subtraction (scores are ~N(0,1.3); fp32 exp cannot
    overflow; block-0 masking is a 0/1 multiply on exp with a per-core mask).
  * Score passes run before kv passes in each group, so the softmax sums and
    reciprocal are off the critical path.
  * RMSNorm channel reduction via tiny accumulating matmuls with
    lhsT=pooled^2 chunks -> var lands with partition=block, matching the
    Hadamard output layout; scale applied per-partition after
    the final Hadamard matmul (everything in between is linear).
  * Hadamard matmuls in fp16 with H rows +-1 (exact); the 1/sqrt(512)
    normalization folds into the RMSNorm scale. 1 cycle/row vs 4 for fp32.
  * j-groups in order (3,0,1,2): the rope fix-up on chunk 3 overlaps later
    groups; the very last group is split in half-N so its pooling chain
    overlaps the second half's matmuls.
"""

import os
import numpy as np
import ml_dtypes

import concourse.bass as bass
import concourse.bacc as bacc
import concourse.mybir as mybir
from concourse.tile import TileContext
from concourse.bass_utils import run_bass_kernel_spmd

BF16 = ml_dtypes.bfloat16
F32 = mybir.dt.float32
F16 = mybir.dt.float16
BF = mybir.dt.bfloat16

N_CORES = 8
DIM = 4096
ROWS = 2048         # own rows per core
MCH = 4             # m-chunks per core
MROWS = 512         # rows per m-chunk
NBLK = 128          # blocks per m-chunk
DCH = 32            # d chunks of 128
WSUB = 4            # sub-DMAs for the very first weight tile
XQ = 4              # x^T quarter-DMAs per m-chunk (8 d-chunks each)
# o-chunks 0..3 kv-first(prev), 4..7 kv-second(cur), 8..11 sc-first, 12..15 sc-second
FIRST_HALF = (0, 1, 2, 3, 8, 9, 10, 11)

_CACHE = {}


def _build():
    nc = bacc.Bacc("TRN2", target_bir_lowering=False, debug=False,
                   num_devices=N_CORES)
    xs2 = nc.dram_tensor("xs2", [MCH, 128, DCH, 528], BF, kind="ExternalInput")
    wp = nc.dram_tensor("wp", [16, 128, DCH, 128], BF, kind="ExternalInput")
    ape_d = nc.dram_tensor("ape_t", [128, 32], F32, kind="ExternalInput")
    cos_d = nc.dram_tensor("cos_t", [128, 512], F16, kind="ExternalInput")
    sin_d = nc.dram_tensor("sin_t", [128, 512], F16, kind="ExternalInput")
    psw_d = nc.dram_tensor("psw", [128, 128], F16, kind="ExternalInput")
    h_d = nc.dram_tensor("hmat", [128, 4, 512], F16, kind="ExternalInput")
    zmask_d = nc.dram_tensor("zmask", [128, 1], F32, kind="ExternalInput")
    out_d = nc.dram_tensor("out", [4 * NBLK, 512], F32, kind="ExternalOutput")

    X = mybir.AxisListType.X
    AF = mybir.ActivationFunctionType

    with TileContext(nc) as tc:
        with (
            tc.tile_pool(name="const", bufs=1) as constp,
            tc.tile_pool(name="xt", bufs=2) as xtp,
            tc.tile_pool(name="wt", bufs=2) as wtp,
            tc.tile_pool(name="sb", bufs=2) as sbp,
            tc.tile_pool(name="pl", bufs=2) as plp,
            tc.tile_pool(name="sm", bufs=2) as smp,
            tc.tile_pool(name="osb", bufs=1) as outp,
            tc.tile_pool(name="proj", bufs=4, space="PSUM") as projp,
            tc.tile_pool(name="had", bufs=1, space="PSUM") as hadp,
            tc.tile_pool(name="aux", bufs=1, space="PSUM") as auxp,
        ):
            # ---- constants (scalar queue: off the main input queue) ----
            ape_sb = constp.tile([128, 32], F32, tag="ape")
            nc.scalar.dma_start(out=ape_sb[:], in_=ape_d[:, :])
            cos_sb = constp.tile([128, 512], F16, tag="cos")
            nc.scalar.dma_start(out=cos_sb[:], in_=cos_d[:, :])
            sin_sb = constp.tile([128, 512], F16, tag="sin")
            nc.scalar.dma_start(out=sin_sb[:], in_=sin_d[:, :])
            psw_sb = constp.tile([128, 128], F16, tag="psw")
            nc.scalar.dma_start(out=psw_sb[:], in_=psw_d[:, :])
            h_sb = constp.tile([128, 4, 512], F16, tag="h")
            nc.scalar.dma_start(out=h_sb[:], in_=h_d[:, :, :])
            zmask_sb = constp.tile([128, 1], F32, tag="zmask")
            nc.scalar.dma_start(out=zmask_sb[:], in_=zmask_d[:, :])
            ones_sb = constp.tile([128, 1], F16, tag="ones")
            nc.vector.memset(ones_sb[:], 1.0)
            eps_sb = constp.tile([128, 1], F32, tag="eps")
            nc.vector.memset(eps_sb[:], 512e-6)

            def wdma(oc, t):
                w = wtp.tile([128, DCH, 128], BF, tag=f"w{t}")
                nc.sync.dma_start(out=w[:], in_=wp[oc])
                return w

            def xdma(mch, q):
                xt = xtp.tile([128, 8, 528], BF, tag=f"xq{q}")
                nc.sync.dma_start(out=xt[:], in_=xs2[mch, :, 8 * q:8 * (q + 1), :])
                return xt

            # ---- mch0 startup: hand-ordered single-queue DMA interleave so
            # the PE ramp is DMA-feasibility-limited. The first two passes
            # (t2=oc11, t3=oc15) run PAIRED per d-chunk, consuming at the
            # x^T delivery rate; their weight tiles are split in 4 and
            # interleaved with x^T eighth-DMAs at the head of the queue.
            w0subs, w1subs, xq0 = [], [], []

            def wsub(lst, oc, tagp, s):
                w = constp.tile([128, 8, 128], BF, tag=f"{tagp}{s}")
                nc.sync.dma_start(out=w[:], in_=wp[oc, :, 8 * s:8 * (s + 1), :])
                lst.append(w)

            def xq0half(q, h):
                if h == 0:
                    xt = xtp.tile([128, 8, 528], BF, tag=f"xq{q}")
                    xq0.append(xt)
                nc.sync.dma_start(
                    out=xq0[q][:, 4 * h:4 * (h + 1), :],
                    in_=xs2[0, :, 8 * q + 4 * h:8 * q + 4 * (h + 1), :])

            wsub(w0subs, 11, "w0s", 0)
            wsub(w1subs, 15, "w1s", 0)
            xq0half(0, 0)
            wsub(w0subs, 11, "w0s", 1)
            wsub(w1subs, 15, "w1s", 1)
            xq0half(0, 1)
            wsub(w0subs, 11, "w0s", 2)
            wsub(w1subs, 15, "w1s", 2)
            xq0half(1, 0)
            wsub(w0subs, 11, "w0s", 3)
            wsub(w1subs, 15, "w1s", 3)
            xq0half(1, 1)
            xq0half(2, 0)
            xq0half(2, 1)
            xq0half(3, 0)
            xq0half(3, 1)
            wj3 = {2: None, 3: None}
            wj3[0] = wdma(3, 0)
            wj3[1] = wdma(7, 1)

            for mch in range(MCH):
                # x^T quarters: [128(dpart), 8(dchunk), 528(m)] bf16; m slot
                # s <-> own row 512*mch + s - 16; slots 12..15 = halo rows.
                if mch == 0:
                    xqs = xq0
                else:
                    xqs = [xdma(mch, q) for q in range(XQ)]

                pooled = plp.tile([128, 4, NBLK], F16, tag="pooled")
                sq = plp.tile([128, 4, NBLK], F16, tag="sq")

                def emit_group(j, b0, nblk, wtiles, pstiles=None):
                    """Projection + pooling for chunk-group j, blocks
                    [b0, b0+nblk) of this m-chunk. wtiles: {t: tile or None}.
                    For half-N groups, pstiles carries full-width psum tiles
                    shared by both halves (separate column ranges)."""
                    nrows = 4 * nblk
                    m0 = 4 * b0
                    group = {}
                    paired = wtiles.get(2) is None
                    done_mm = set()
                    # score passes (t2,t3) first, kv passes (t0,t1) last: the
                    # softmax sums + reciprocal run during the kv matmuls
                    for t in (2, 3, 0, 1):
                        oc = j + 4 * t
                        if wtiles.get(t) is None:
                            subs = w0subs if t == 2 else w1subs

                            def wslice(d, subs=subs):
                                return subs[d // 8][:, d % 8, :]
                        else:
                            def wslice(d, w=wtiles[t]):
                                return w[:, d, :]
                        if pstiles is None:
                            ps_t = projp.tile([128, nrows], F32, tag="proj")
                            ps = ps_t[:]
                        else:
                            ps = pstiles[t][:, m0:m0 + nrows]
                        off = (12 if oc in FIRST_HALF else 16) + m0
                        if t not in done_mm:
                            if paired and t == 2:
                                # interleave t2/t3 per d-chunk: combined
                                # consumption rate matches the x^T delivery
                                ps3_t = projp.tile([128, nrows], F32,
                                                   tag="proj")
                                ps3 = ps3_t[:]
                                off3 = (12 if (j + 12) in FIRST_HALF
                                        else 16) + m0
                                for d in range(DCH):
                                    rhs = xqs[d // 8][:, d % 8, :]
                                    nc.tensor.matmul(
                                        ps, lhsT=wslice(d),
                                        rhs=rhs[:, off:off + nrows],
                                        start=(d == 0), stop=(d == DCH - 1),
                                        skip_group_check=True)
                                    nc.tensor.matmul(
                                        ps3,
                                        lhsT=w1subs[d // 8][:, d % 8, :],
                                        rhs=rhs[:, off3:off3 + nrows],
                                        start=(d == 0), stop=(d == DCH - 1),
                                        skip_group_check=True)
                                group["ps3"] = ps3
                                done_mm.add(3)
                            else:
                                for d in range(DCH):
                                    nc.tensor.matmul(
                                        ps,
                                        lhsT=wslice(d),
                                        rhs=xqs[d // 8][:, d % 8,
                                                        off:off + nrows],
                                        start=(d == 0),
                                        stop=(d == DCH - 1),
                                        skip_group_check=(pstiles is not None),
                                    )
                        else:
                            ps = group.pop("ps3")
                        if t < 2:
                            # kv chunk: PSUM -> SBUF with ape bias added
                            kv = sbp.tile([128, nrows], F32, tag=f"kv{t}_{nblk}")
                            a = oc  # ape chunk = kv o-chunk (0..7)
                            ape_ap = (ape_sb[:, 4 * a:4 * a + 4]
                                      .unsqueeze(1).to_broadcast((128, nblk, 4)))
                            nc.vector.tensor_add(
                                kv[:].rearrange("p (b r) -> p b r", r=4),
                                ps.rearrange("p (b r) -> p b r", r=4),
                                ape_ap,
                            )
                            group[f"kv{t}"] = kv
                        else:
                            # score chunk: e = exp(psum) straight to SBUF
                            e = sbp.tile([128, nrows], F32, tag=f"e{t}_{nblk}")
                            nc.scalar.activation(e[:], ps, AF.Exp)
                            if mch == 0 and b0 == 0 and t == 2:
                                # block-0 of even cores: zero the 4 prev-window
                                # weights (zmask = 0 even / 1 odd)
                                nc.vector.tensor_scalar_mul(
                                    e[:, 0:4], e[:, 0:4], zmask_sb[:, 0:1])
                            group[f"e{t}"] = e

                        if t == 3:
                            # softmax denominator, as soon as both e's exist
                            e1, e2 = group["e2"], group["e3"]
                            s1 = smp.tile([128, nblk], F32, tag=f"s1_{nblk}")
                            nc.vector.reduce_sum(
                                s1[:], e1[:].rearrange("p (b r) -> p b r", r=4),
                                axis=X)
                            s2 = smp.tile([128, nblk], F32, tag=f"s2_{nblk}")
                            nc.vector.reduce_sum(
                                s2[:], e2[:].rearrange("p (b r) -> p b r", r=4),
                                axis=X)
                            ssum = smp.tile([128, nblk], F32, tag=f"ss_{nblk}")
                            nc.vector.tensor_add(ssum[:], s1[:], s2[:])
                            rinv = smp.tile([128, nblk], F32, tag=f"ri_{nblk}")
                            nc.vector.reciprocal(rinv[:], ssum[:])
                            group["rinv"] = rinv
                        if t == 0:
                            # first kv ready: weighted sum of the prev-window
                            pm = sbp.tile([128, nrows], F32, tag=f"pm_{nblk}")
                            nc.vector.tensor_mul(pm[:], group["e2"][:],
                                                 group["kv0"][:])
                            q1 = smp.tile([128, nblk], F32, tag=f"q1_{nblk}")
                            nc.vector.reduce_sum(
                                q1[:], pm[:].rearrange("p (b r) -> p b r", r=4),
                                axis=X)
                            group["q1"] = q1

                    pm2 = sbp.tile([128, nrows], F32, tag=f"pm2_{nblk}")
                    nc.vector.tensor_mul(pm2[:], group["e3"][:], group["kv1"][:])
                    q2 = smp.tile([128, nblk], F32, tag=f"q2_{nblk}")
                    nc.vector.reduce_sum(
                        q2[:], pm2[:].rearrange("p (b r) -> p b r", r=4), axis=X)
                    qsum = smp.tile([128, nblk], F32, tag=f"qs_{nblk}")
                    nc.vector.tensor_add(qsum[:], group["q1"][:], q2[:])
                    nc.vector.tensor_mul(pooled[:, j, b0:b0 + nblk], qsum[:],
                                         group["rinv"][:])
                    # squared copy for the RMSNorm variance (pre-rope: rope is
                    # norm-preserving; the norm scale is applied at the end)
                    nc.scalar.activation(sq[:, j, b0:b0 + nblk],
                                         pooled[:, j, b0:b0 + nblk], AF.Square)

                # j order: chunk 3 first so its rope fix-up overlaps later
                # groups; last group of the last m-chunk split in half-N so
                # its pooling chain overlaps the second half's matmuls.
                for j in (3, 0, 1, 2):
                    if mch == 0 and j == 3:
                        wtiles = wj3
                    else:
                        wtiles = {t: wdma(j + 4 * t, t) for t in (2, 3, 0, 1)}
                    if mch == MCH - 1 and j == 2:
                        pst = {}
                        for t in (2, 3, 0, 1):
                            ps_full = projp.tile([128, MROWS], F32, tag="proj")
                            pst[t] = ps_full
                        emit_group(j, 0, NBLK // 2, wtiles, pst)
                        emit_group(j, NBLK // 2, NBLK // 2, wtiles, pst)
                    else:
                        emit_group(j, 0, NBLK, wtiles)

                    if j == 3:
                        # rope on chunk 3 (ch 384..511; rows 64.. are rope),
                        # right after its pooling so it overlaps group 0
                        sw_ps = auxp.tile([128, NBLK], F32, tag="swap")
                        nc.tensor.matmul(sw_ps[:], lhsT=psw_sb[:],
                                         rhs=pooled[:, 3, :],
                                         start=True, stop=True)
                        cslice = cos_sb[:, mch * NBLK:(mch + 1) * NBLK]
                        sslice = sin_sb[:, mch * NBLK:(mch + 1) * NBLK]
                        tmpc = smp.tile([128, NBLK], F16, tag="tmpc")
                        nc.vector.tensor_mul(tmpc[:], pooled[:, 3, :], cslice)
                        tmps = smp.tile([128, NBLK], F16, tag="tmps")
                        nc.vector.tensor_mul(tmps[:], sw_ps[:], sslice)
                        nc.vector.tensor_add(pooled[:, 3, :], tmpc[:], tmps[:])

                # ---- RMSNorm stats: var[blk] via accumulating matmuls with
                # lhsT=sq chunks -> PSUM [128(blk), 1]; then scale = Rsqrt
                var_ps = auxp.tile([128, 1], F32, tag="var")
                for jj in range(4):
                    nc.tensor.matmul(var_ps[:], lhsT=sq[:, jj, :],
                                     rhs=ones_sb[:, 0:1],
                                     start=(jj == 0), stop=(jj == 3))
                # scale = 1/sqrt(var + 512*eps) = (1/sqrt(512)) / sqrt(
                # var/512 + eps); the 1/sqrt(512) folds the Hadamard
                # normalization (H rows are +-1 on device)
                sd_col = smp.tile([128, 1], F32, tag="sd_col")
                nc.scalar.activation(sd_col[:], var_ps[:], AF.Sqrt,
                                     scale=1.0, bias=eps_sb[:, 0:1])
                scale_col = smp.tile([128, 1], F32, tag="scale_col")
                nc.vector.reciprocal(scale_col[:], sd_col[:])

                # ---- Hadamard: out[blk, c'] = sum_c pooled[c, blk] H[c, c']
                # (fp16 operands: 1 cycle/row vs 4 for fp32; H rows +-1 exact)
                had_ps = hadp.tile([128, 512], F32, tag="had")
                for jj in range(4):
                    nc.tensor.matmul(had_ps[:],
                                     lhsT=pooled[:, jj, :],
                                     rhs=h_sb[:, jj, :],
                                     start=(jj == 0), stop=(jj == 3))
                out_sb = outp.tile([128, 512], F32, tag="out")
                nc.scalar.activation(out_sb[:], had_ps[:], AF.Copy,
                                     scale=scale_col[:, 0:1])
                nc.gpsimd.dma_start(
                    out=out_d[mch * NBLK:(mch + 1) * NBLK, :], in_=out_sb[:])
    nc.compile()
    return nc


def _prep_shared(W_kv, W_gate, ape, norm_w, H):
    W = np.concatenate([W_kv, W_gate], axis=0).astype(np.float32)  # [2048, 4096]
    Wb = W.astype(BF16)
    wp = np.ascontiguousarray(
        Wb.T.reshape(DCH, 128, 16, 128).transpose(2, 1, 0, 3))  # [16,128,32,128]
    ape_t = np.ascontiguousarray(
        ape.astype(np.float32).T.reshape(8, 128, 4).transpose(1, 0, 2)
    ).reshape(128, 32)
    psw = np.zeros((128, 128), np.float16)
    idx = np.arange(64)
    psw[idx, idx] = 1.0
    k2 = np.arange(0, 64, 2)
    psw[64 + k2 + 1, 64 + k2] = 1.0
    psw[64 + k2, 64 + k2 + 1] = 1.0
    hm = np.ascontiguousarray(
        (norm_w.astype(np.float32)[:, None] * H.astype(np.float32)
         * np.sqrt(512.0, dtype=np.float32))
        .reshape(4, 128, 512).transpose(1, 0, 2)).astype(np.float16)
    return wp, ape_t, psw, hm


def _hadamard(n):
    h = np.array([[1.0]], dtype=np.float32)
    while h.shape[0] < n:
        h = np.block([[h, h], [h, -h]])
    return (h / np.sqrt(n)).astype(np.float32)


def _make_in_maps(x, W_kv, W_gate, ape, norm_w, freqs_cis):
    b, s, _ = x.shape
    H = _hadamard(512)
    wp, ape_t, psw, hm = _prep_shared(W_kv, W_gate, ape, norm_w, H)

    # truncate-to-bf16 (hi-16 planes of the f32 words) and transpose once
    xh = x.reshape(b * s, DIM).view(BF16)[:, 1::2]
    xT = np.ascontiguousarray(xh.T)  # [4096, 16384]
    fr = freqs_cis[:, :, 0]  # [nb, 32]
    fi = freqs_cis[:, :, 1]

    in_maps = []
    for c in range(N_CORES):
        batch, half = c // 2, c % 2
        R0 = batch * s + half * ROWS
        xs = np.zeros((DIM, 16 + ROWS), BF16)
        xs[:, 16:] = xT[:, R0:R0 + ROWS]
        if half == 1:
            xs[:, :16] = xT[:, R0 - 16:R0]
        # per-m-chunk windows, re-tiled so every DMA descriptor is a
        # contiguous 8.4KB run: [mch][128 dpart][32 dchunk][528 m]
        xs2 = np.empty((MCH, 128, DCH, 528), BF16)
        for m in range(MCH):
            win = xs[:, 512 * m:512 * m + 528]
            xs2[m] = win.reshape(DCH, 128, 528).transpose(1, 0, 2)

        g0 = half * 512
        bi = np.arange(g0, g0 + 512)
        cos_t = np.zeros((128, 512), np.float16)
        cos_t[:64] = 1.0
        cos_t[64:] = np.repeat(fr[bi].T, 2, axis=0).astype(np.float16)
        sin_t = np.zeros((128, 512), np.float16)
        st = np.repeat(fi[bi].T, 2, axis=0)
        st[0::2] *= -1.0
        sin_t[64:] = st.astype(np.float16)

        zmask = np.full((128, 1), 0.0 if half == 0 else 1.0, np.float32)
        in_maps.append({
            "xs2": xs2, "wp": wp, "ape_t": ape_t,
            "cos_t": cos_t, "sin_t": sin_t, "psw": psw,
            "hmat": hm, "zmask": zmask,
        })
    return in_maps


def kernel(x, W_kv, W_gate, ape, norm_w, freqs_cis, start_pos=0):
    x = np.asarray(x, dtype=np.float32)
    W_kv = np.asarray(W_kv, dtype=np.float32)
    W_gate = np.asarray(W_gate, dtype=np.float32)
    ape = np.asarray(ape, dtype=np.float32)
    norm_w = np.asarray(norm_w, dtype=np.float32)
    freqs_cis = np.asarray(freqs_cis, dtype=np.float32)

    b, s, _ = x.shape
    nb = s // 4
    assert (b, s) == (4, 4096), (b, s)

    if "nc" not in _CACHE:
        _CACHE["nc"] = _build()
    nc = _CACHE["nc"]

    in_maps = _make_in_maps(x, W_kv, W_gate, ape, norm_w, freqs_cis)

    trace = os.environ.get("KERNEL_TRACE", "") not in ("", "0")
    res = run_bass_kernel_spmd(nc, in_maps, core_ids=list(range(N_CORES)),
                               trace=trace)
    kernel.last_results = res
    out = np.concatenate([res.results[c]["out"] for c in range(N_CORES)], axis=0)
    return np.ascontiguousarray(out.reshape(b, nb, 512))


# revision 17
# speedup vs baseline: 1.2666x; 1.0040x over previous
"""Trainium2 Bass kernel for nn_Compressor (sparse_attention block compressor).

Math (reference):
  proj = x @ [W_kv; W_gate]^T            # [b*s, 2048]
  kv   = proj[:, :1024] + ape[s%4]       # blockwise (RATIO=4) abs-pos bias
  sc   = proj[:, 1024:]
  window(blk) = {prev blk rows, ch 0:512} + {cur blk rows, ch 512:1024}
  pooled[blk, c] = softmax-gated channelwise pool over the 8-entry window
  out = (RMSNorm(pooled) -> rope on ch 448:512) @ H  (512x512 Hadamard)

Distribution: 8 cores, data-parallel over (batch, seq-half). Each core owns
2048 seq rows = 512 blocks; the 1-block halo is handled by shifting the
matmul rhs window by 4 rows (the x^T input carries 16 halo rows per m-chunk).

Key implementation tricks:
  * x^T in bf16 obtained host-side by truncating f32 to the hi-16 planes,
    then re-tiled per m-chunk to [128(dpart), 32(dchunk), 528(m)] so each
    DMA descriptor covers a 8.4KB contiguous run (DMA descriptor rate, not
    bandwidth, is the scarce resource for thin transfers).
  * All input DMAs share the sync queue in a hand-chosen order (first weight
    tile split in 4 -> mch0 x^T quarters interleaved with the next weight
    tiles), so the PE ramps at the DMA-feasibility limit and cross-queue
    descriptor round-robin can't starve anything.
  * Projections: W^T tiles stationary (lhsT), x^T moving -> PSUM layout
    [channels(part), m(free)], so the whole softmax pooling is free-axis
    DVE/ACT work and the halo is a free-axis slice offset.
  * Softmax without max-subtraction (scores are ~N(0,1.3); fp32 exp cannot
    overflow; block-0 masking is a 0/1 multiply on exp with a per-core mask).
  * Score passes run before kv passes in each group, so the softmax sums and
    reciprocal are off the critical path.
  * RMSNorm channel reduction via tiny accumulating matmuls with
    lhsT=pooled^2 chunks -> var lands with partition=block, matching the
    Hadamard output layout; scale applied per-partition after
    the final Hadamard matmul (everything in between is linear).
  * Hadamard matmuls in fp16 with H rows +-1 (exact); the 1/sqrt(512)
    normalization folds into the RMSNorm scale. 1 cycle/row vs 4 for fp32.
  * j-groups in order (3,0,1,2): the rope fix-up on chunk 3 overlaps later
    groups; the very last group is split in half-N so its pooling chain
    overlaps the second half's matmuls.
"""

import os
import numpy as np
import ml_dtypes

import concourse.bass as bass
import concourse.bacc as bacc
import concourse.mybir as mybir
from concourse.tile import TileContext
from concourse.bass_utils import run_bass_kernel_spmd

BF16 = ml_dtypes.bfloat16
F32 = mybir.dt.float32
F16 = mybir.dt.float16
BF = mybir.dt.bfloat16

N_CORES = 8
DIM = 4096
ROWS = 2048         # own rows per core
MCH = 4             # m-chunks per core
MROWS = 512         # rows per m-chunk
NBLK = 128          # blocks per m-chunk
DCH = 32            # d chunks of 128
WSUB = 4            # sub-DMAs for the very first weight tile
XQ = 4              # x^T quarter-DMAs per m-chunk (8 d-chunks each)
# o-chunks 0..3 kv-first(prev), 4..7 kv-second(cur), 8..11 sc-first, 12..15 sc-second
FIRST_HALF = (0, 1, 2, 3, 8, 9, 10, 11)

_CACHE = {}


def _build():
    nc = bacc.Bacc("TRN2", target_bir_lowering=False, debug=False,
                   num_devices=N_CORES)
    xs2 = nc.dram_tensor("xs2", [MCH, 128, DCH, 528], BF, kind="ExternalInput")
    wp = nc.dram_tensor("wp", [16, 128, DCH, 128], BF, kind="ExternalInput")
    ape_d = nc.dram_tensor("ape_t", [128, 32], F32, kind="ExternalInput")
    cos_d = nc.dram_tensor("cos_t", [128, 512], F16, kind="ExternalInput")
    sin_d = nc.dram_tensor("sin_t", [128, 512], F16, kind="ExternalInput")
    psw_d = nc.dram_tensor("psw", [128, 128], F16, kind="ExternalInput")
    h_d = nc.dram_tensor("hmat", [128, 4, 512], F16, kind="ExternalInput")
    zmask_d = nc.dram_tensor("zmask", [128, 1], F32, kind="ExternalInput")
    out_d = nc.dram_tensor("out", [4 * NBLK, 512], F32, kind="ExternalOutput")

    X = mybir.AxisListType.X
    AF = mybir.ActivationFunctionType

    with TileContext(nc) as tc:
        with (
            tc.tile_pool(name="const", bufs=1) as constp,
            tc.tile_pool(name="xt", bufs=2) as xtp,
            tc.tile_pool(name="wt", bufs=2) as wtp,
            tc.tile_pool(name="sb", bufs=2) as sbp,
            tc.tile_pool(name="pl", bufs=2) as plp,
            tc.tile_pool(name="sm", bufs=2) as smp,
            tc.tile_pool(name="osb", bufs=1) as outp,
            tc.tile_pool(name="proj", bufs=4, space="PSUM") as projp,
            tc.tile_pool(name="had", bufs=1, space="PSUM") as hadp,
            tc.tile_pool(name="aux", bufs=1, space="PSUM") as auxp,
        ):
            # ---- constants (scalar queue: off the main input queue) ----
            ape_sb = constp.tile([128, 32], F32, tag="ape")
            nc.scalar.dma_start(out=ape_sb[:], in_=ape_d[:, :])
            cos_sb = constp.tile([128, 512], F16, tag="cos")
            nc.scalar.dma_start(out=cos_sb[:], in_=cos_d[:, :])
            sin_sb = constp.tile([128, 512], F16, tag="sin")
            nc.scalar.dma_start(out=sin_sb[:], in_=sin_d[:, :])
            psw_sb = constp.tile([128, 128], F16, tag="psw")
            nc.scalar.dma_start(out=psw_sb[:], in_=psw_d[:, :])
            h_sb = constp.tile([128, 4, 512], F16, tag="h")
            nc.scalar.dma_start(out=h_sb[:], in_=h_d[:, :, :])
            zmask_sb = constp.tile([128, 1], F32, tag="zmask")
            nc.scalar.dma_start(out=zmask_sb[:], in_=zmask_d[:, :])
            ones_sb = constp.tile([128, 1], F16, tag="ones")
            nc.vector.memset(ones_sb[:], 1.0)
            eps_sb = constp.tile([128, 1], F32, tag="eps")
            nc.vector.memset(eps_sb[:], 512e-6)

            def wdma(oc, t):
                w = wtp.tile([128, DCH, 128], BF, tag=f"w{t}")
                nc.sync.dma_start(out=w[:], in_=wp[oc])
                return w

            def xdma(mch, q):
                xt = xtp.tile([128, 8, 528], BF, tag=f"xq{q}")
                nc.sync.dma_start(out=xt[:], in_=xs2[mch, :, 8 * q:8 * (q + 1), :])
                return xt

            # ---- mch0 startup: hand-ordered single-queue DMA interleave so
            # the PE ramp is DMA-feasibility-limited. The first two passes
            # (t2=oc11, t3=oc15) run PAIRED per d-chunk, consuming at the
            # x^T delivery rate; their weight tiles are split in 4 and
            # interleaved with x^T eighth-DMAs at the head of the queue.
            w0subs, w1subs, xq0 = [], [], []

            def wsub(lst, oc, tagp, s):
                w = constp.tile([128, 8, 128], BF, tag=f"{tagp}{s}")
                nc.sync.dma_start(out=w[:], in_=wp[oc, :, 8 * s:8 * (s + 1), :])
                lst.append(w)

            def xq0half(q, h):
                if h == 0:
                    xt = xtp.tile([128, 8, 528], BF, tag=f"xq{q}")
                    xq0.append(xt)
                nc.sync.dma_start(
                    out=xq0[q][:, 4 * h:4 * (h + 1), :],
                    in_=xs2[0, :, 8 * q + 4 * h:8 * q + 4 * (h + 1), :])

            wsub(w0subs, 11, "w0s", 0)
            wsub(w1subs, 15, "w1s", 0)
            xq0half(0, 0)
            wsub(w0subs, 11, "w0s", 1)
            wsub(w1subs, 15, "w1s", 1)
            xq0half(0, 1)
            wsub(w0subs, 11, "w0s", 2)
            wsub(w1subs, 15, "w1s", 2)
            xq0half(1, 0)
            wsub(w0subs, 11, "w0s", 3)
            wsub(w1subs, 15, "w1s", 3)
            xq0half(1, 1)
            xq0half(2, 0)
            xq0half(2, 1)
            xq0half(3, 0)
            xq0half(3, 1)
            wj3 = {2: None, 3: None}
            wj3[0] = wdma(3, 0)
            wj3[1] = wdma(7, 1)

            for mch in range(MCH):
                # x^T quarters: [128(dpart), 8(dchunk), 528(m)] bf16; m slot
                # s <-> own row 512*mch + s - 16; slots 12..15 = halo rows.
                if mch == 0:
                    xqs = xq0
                else:
                    xqs = [xdma(mch, q) for q in range(XQ)]

                pooled = plp.tile([128, 4, NBLK], F16, tag="pooled")
                sq = plp.tile([128, 4, NBLK], F16, tag="sq")

                def emit_group(j, b0, nblk, wtiles, pstiles=None):
                    """Projection + pooling for chunk-group j, blocks
                    [b0, b0+nblk) of this m-chunk. wtiles: {t: tile or None}.
                    For half-N groups, pstiles carries full-width psum tiles
                    shared by both halves (separate column ranges)."""
                    nrows = 4 * nblk
                    m0 = 4 * b0
                    group = {}
                    paired = wtiles.get(2) is None
                    done_mm = set()
                    # score passes (t2,t3) first, kv passes (t0,t1) last: the
                    # softmax sums + reciprocal run during the kv matmuls
                    for t in (2, 3, 0, 1):
                        oc = j + 4 * t
                        if wtiles.get(t) is None:
                            subs = w0subs if t == 2 else w1subs

                            def wslice(d, subs=subs):
                                return subs[d // 8][:, d % 8, :]
                        else:
                            def wslice(d, w=wtiles[t]):
                                return w[:, d, :]
                        if pstiles is None:
                            ps_t = projp.tile([128, nrows], F32, tag="proj")
                            ps = ps_t[:]
                        else:
                            ps = pstiles[t][:, m0:m0 + nrows]
                        off = (12 if oc in FIRST_HALF else 16) + m0
                        if t not in done_mm:
                            if paired and t == 2:
                                # interleave t2/t3 per d-chunk: combined
                                # consumption rate matches the x^T delivery
                                ps3_t = projp.tile([128, nrows], F32,
                                                   tag="proj")
                                ps3 = ps3_t[:]
                                off3 = (12 if (j + 12) in FIRST_HALF
                                        else 16) + m0
                                for d in range(DCH):
                                    rhs = xqs[d // 8][:, d % 8, :]
                                    nc.tensor.matmul(
                                        ps, lhsT=wslice(d),
                                        rhs=rhs[:, off:off + nrows],
                                        start=(d == 0), stop=(d == DCH - 1),
                                        skip_group_check=True)
                                    nc.tensor.matmul(
                                        ps3,
                                        lhsT=w1subs[d // 8][:, d % 8, :],
                                        rhs=rhs[:, off3:off3 + nrows],
                                        start=(d == 0), stop=(d == DCH - 1),
                                        skip_group_check=True)
                                group["ps3"] = ps3
                                done_mm.add(3)
                            else:
                                for d in range(DCH):
                                    nc.tensor.matmul(
                                        ps,
                                        lhsT=wslice(d),
                                        rhs=xqs[d // 8][:, d % 8,
                                                        off:off + nrows],
                                        start=(d == 0),
                                        stop=(d == DCH - 1),
                                        skip_group_check=(pstiles is not None),
                                    )
                        else:
                            ps = group.pop("ps3")
                        if t < 2:
                            # kv chunk: PSUM -> SBUF with ape bias added
                            kv = sbp.tile([128, nrows], F32, tag=f"kv{t}_{nblk}")
                            a = oc  # ape chunk = kv o-chunk (0..7)
                            ape_ap = (ape_sb[:, 4 * a:4 * a + 4]
                                      .unsqueeze(1).to_broadcast((128, nblk, 4)))
                            nc.vector.tensor_add(
                                kv[:].rearrange("p (b r) -> p b r", r=4),
                                ps.rearrange("p (b r) -> p b r", r=4),
                                ape_ap,
                            )
                            group[f"kv{t}"] = kv
                        else:
                            # score chunk: e = exp(psum) straight to SBUF
                            e = sbp.tile([128, nrows], F32, tag=f"e{t}_{nblk}")
                            nc.scalar.activation(e[:], ps, AF.Exp)
                            if mch == 0 and b0 == 0 and t == 2:
                                # block-0 of even cores: zero the 4 prev-window
                                # weights (zmask = 0 even / 1 odd)
                                nc.vector.tensor_scalar_mul(
                                    e[:, 0:4], e[:, 0:4], zmask_sb[:, 0:1])
                            group[f"e{t}"] = e

                        if t == 3:
                            # softmax denominator, as soon as both e's exist
                            e1, e2 = group["e2"], group["e3"]
                            s1 = smp.tile([128, nblk], F32, tag=f"s1_{nblk}")
                            nc.vector.reduce_sum(
                                s1[:], e1[:].rearrange("p (b r) -> p b r", r=4),
                                axis=X)
                            s2 = smp.tile([128, nblk], F32, tag=f"s2_{nblk}")
                            nc.vector.reduce_sum(
                                s2[:], e2[:].rearrange("p (b r) -> p b r", r=4),
                                axis=X)
                            ssum = smp.tile([128, nblk], F32, tag=f"ss_{nblk}")
                            nc.vector.tensor_add(ssum[:], s1[:], s2[:])
                            rinv = smp.tile([128, nblk], F32, tag=f"ri_{nblk}")
                            nc.vector.reciprocal(rinv[:], ssum[:])
                            group["rinv"] = rinv
                        if t == 0:
                            # first kv ready: weighted sum of the prev-window
                            pm = sbp.tile([128, nrows], F32, tag=f"pm_{nblk}")
                            nc.vector.tensor_mul(pm[:], group["e2"][:],
                                                 group["kv0"][:])
                            q1 = smp.tile([128, nblk], F32, tag=f"q1_{nblk}")
                            nc.vector.reduce_sum(
                                q1[:], pm[:].rearrange("p (b r) -> p b r", r=4),
                                axis=X)
                            group["q1"] = q1

                    pm2 = sbp.tile([128, nrows], F32, tag=f"pm2_{nblk}")
                    nc.vector.tensor_mul(pm2[:], group["e3"][:], group["kv1"][:])
                    q2 = smp.tile([128, nblk], F32, tag=f"q2_{nblk}")
                    nc.vector.reduce_sum(
                        q2[:], pm2[:].rearrange("p (b r) -> p b r", r=4), axis=X)
                    qsum = smp.tile([128, nblk], F32, tag=f"qs_{nblk}")
                    nc.vector.tensor_add(qsum[:], group["q1"][:], q2[:])
                    nc.vector.tensor_mul(pooled[:, j, b0:b0 + nblk], qsum[:],
                                         group["rinv"][:])
                    # squared copy for the RMSNorm variance (pre-rope: rope is
                    # norm-preserving; the norm scale is applied at the end)
                    nc.scalar.activation(sq[:, j, b0:b0 + nblk],
                                         pooled[:, j, b0:b0 + nblk], AF.Square)

                # j order: chunk 3 first so its rope fix-up overlaps later
                # groups; last group of the last m-chunk split in half-N so
                # its pooling chain overlaps the second half's matmuls.
                for j in (3, 0, 1, 2):
                    if mch == 0 and j == 3:
                        wtiles = wj3
                    else:
                        wtiles = {t: wdma(j + 4 * t, t) for t in (2, 3, 0, 1)}
                    if mch == MCH - 1 and j == 2:
                        pst = {}
                        for t in (2, 3, 0, 1):
                            ps_full = projp.tile([128, MROWS], F32, tag="proj")
                            pst[t] = ps_full
                        emit_group(j, 0, NBLK // 2, wtiles, pst)
                        emit_group(j, NBLK // 2, NBLK // 2, wtiles, pst)
                    else:
                        emit_group(j, 0, NBLK, wtiles)

                    if j == 3:
                        # rope on chunk 3 (ch 384..511; rows 64.. are rope),
                        # right after its pooling so it overlaps group 0
                        sw_ps = auxp.tile([128, NBLK], F32, tag="swap")
                        nc.tensor.matmul(sw_ps[:], lhsT=psw_sb[:],
                                         rhs=pooled[:, 3, :],
                                         start=True, stop=True)
                        cslice = cos_sb[:, mch * NBLK:(mch + 1) * NBLK]
                        sslice = sin_sb[:, mch * NBLK:(mch + 1) * NBLK]
                        tmpc = smp.tile([128, NBLK], F16, tag="tmpc")
                        nc.vector.tensor_mul(tmpc[:], pooled[:, 3, :], cslice)
                        tmps = smp.tile([128, NBLK], F16, tag="tmps")
                        nc.vector.tensor_mul(tmps[:], sw_ps[:], sslice)
                        nc.vector.tensor_add(pooled[:, 3, :], tmpc[:], tmps[:])

                # ---- finish (per block-range): RMSNorm var via accumulating
                # matmuls with lhsT=sq chunks -> PSUM [nb(blk), 1]; scale =
                # 1/sqrt(var + 512*eps) (the extra 512 folds the Hadamard
                # normalization: H rows are +-1 on device); Hadamard matmuls
                # in fp16 (1 cycle/row vs 4 for fp32); per-partition scale on
                # the ACT copy-out. For the last m-chunk this runs per
                # half-N, overlapping the second half's projection matmuls.
                var_ps = auxp.tile([128, 1], F32, tag="var")
                had_ps = hadp.tile([128, 512], F32, tag="had")
                out_sb = outp.tile([128, 512], F32, tag="out")

                def finish(lo, hi):
                    for jj in range(4):
                        nc.tensor.matmul(var_ps[lo:hi, 0:1],
                                         lhsT=sq[:, jj, lo:hi],
                                         rhs=ones_sb[:, 0:1],
                                         start=(jj == 0), stop=(jj == 3),
                                         skip_group_check=True)
                    sd_col = smp.tile([128, 1], F32, tag="sd_col")
                    nc.scalar.activation(sd_col[lo:hi, :], var_ps[lo:hi, :],
                                         AF.Sqrt, scale=1.0,
                                         bias=eps_sb[lo:hi, 0:1])
                    scale_col = smp.tile([128, 1], F32, tag="scale_col")
                    nc.vector.reciprocal(scale_col[lo:hi, :], sd_col[lo:hi, :])
                    for jj in range(4):
                        nc.tensor.matmul(had_ps[lo:hi, :],
                                         lhsT=pooled[:, jj, lo:hi],
                                         rhs=h_sb[:, jj, :],
                                         start=(jj == 0), stop=(jj == 3),
                                         skip_group_check=True)
                    nc.scalar.activation(out_sb[lo:hi, :], had_ps[lo:hi, :],
                                         AF.Copy, scale=scale_col[lo:hi, 0:1])
                    nc.gpsimd.dma_start(
                        out=out_d[mch * NBLK + lo:mch * NBLK + hi, :],
                        in_=out_sb[lo:hi, :])

                if mch == MCH - 1:
                    finish(0, NBLK // 2)
                    finish(NBLK // 2, NBLK)
                else:
                    finish(0, NBLK)
    nc.compile()
    return nc


def _prep_shared(W_kv, W_gate, ape, norm_w, H):
    W = np.concatenate([W_kv, W_gate], axis=0).astype(np.float32)  # [2048, 4096]
    Wb = W.astype(BF16)
    wp = np.ascontiguousarray(
        Wb.T.reshape(DCH, 128, 16, 128).transpose(2, 1, 0, 3))  # [16,128,32,128]
    ape_t = np.ascontiguousarray(
        ape.astype(np.float32).T.reshape(8, 128, 4).transpose(1, 0, 2)
    ).reshape(128, 32)
    psw = np.zeros((128, 128), np.float16)
    idx = np.arange(64)
    psw[idx, idx] = 1.0
    k2 = np.arange(0, 64, 2)
    psw[64 + k2 + 1, 64 + k2] = 1.0
    psw[64 + k2, 64 + k2 + 1] = 1.0
    hm = np.ascontiguousarray(
        (norm_w.astype(np.float32)[:, None] * H.astype(np.float32)
         * np.sqrt(512.0, dtype=np.float32))
        .reshape(4, 128, 512).transpose(1, 0, 2)).astype(np.float16)
    return wp, ape_t, psw, hm


def _hadamard(n):
    h = np.array([[1.0]], dtype=np.float32)
    while h.shape[0] < n:
        h = np.block([[h, h], [h, -h]])
    return (h / np.sqrt(n)).astype(np.float32)


def _make_in_maps(x, W_kv, W_gate, ape, norm_w, freqs_cis):
    b, s, _ = x.shape
    H = _hadamard(512)
    wp, ape_t, psw, hm = _prep_shared(W_kv, W_gate, ape, norm_w, H)

    # truncate-to-bf16 (hi-16 planes of the f32 words) and transpose once
    xh = x.reshape(b * s, DIM).view(BF16)[:, 1::2]
    xT = np.ascontiguousarray(xh.T)  # [4096, 16384]
    fr = freqs_cis[:, :, 0]  # [nb, 32]
    fi = freqs_cis[:, :, 1]

    in_maps = []
    for c in range(N_CORES):
        batch, half = c // 2, c % 2
        R0 = batch * s + half * ROWS
        xs = np.zeros((DIM, 16 + ROWS), BF16)
        xs[:, 16:] = xT[:, R0:R0 + ROWS]
        if half == 1:
            xs[:, :16] = xT[:, R0 - 16:R0]
        # per-m-chunk windows, re-tiled so every DMA descriptor is a
        # contiguous 8.4KB run: [mch][128 dpart][32 dchunk][528 m]
        xs2 = np.empty((MCH, 128, DCH, 528), BF16)
        for m in range(MCH):
            win = xs[:, 512 * m:512 * m + 528]
            xs2[m] = win.reshape(DCH, 128, 528).transpose(1, 0, 2)

        g0 = half * 512
        bi = np.arange(g0, g0 + 512)
        cos_t = np.zeros((128, 512), np.float16)
        cos_t[:64] = 1.0
        cos_t[64:] = np.repeat(fr[bi].T, 2, axis=0).astype(np.float16)
        sin_t = np.zeros((128, 512), np.float16)
        st = np.repeat(fi[bi].T, 2, axis=0)
        st[0::2] *= -1.0
        sin_t[64:] = st.astype(np.float16)

        zmask = np.full((128, 1), 0.0 if half == 0 else 1.0, np.float32)
        in_maps.append({
            "xs2": xs2, "wp": wp, "ape_t": ape_t,
            "cos_t": cos_t, "sin_t": sin_t, "psw": psw,
            "hmat": hm, "zmask": zmask,
        })
    return in_maps


def kernel(x, W_kv, W_gate, ape, norm_w, freqs_cis, start_pos=0):
    x = np.asarray(x, dtype=np.float32)
    W_kv = np.asarray(W_kv, dtype=np.float32)
    W_gate = np.asarray(W_gate, dtype=np.float32)
    ape = np.asarray(ape, dtype=np.float32)
    norm_w = np.asarray(norm_w, dtype=np.float32)
    freqs_cis = np.asarray(freqs_cis, dtype=np.float32)

    b, s, _ = x.shape
    nb = s // 4
    assert (b, s) == (4, 4096), (b, s)

    if "nc" not in _CACHE:
        _CACHE["nc"] = _build()
    nc = _CACHE["nc"]

    in_maps = _make_in_maps(x, W_kv, W_gate, ape, norm_w, freqs_cis)

    trace = os.environ.get("KERNEL_TRACE", "") not in ("", "0")
    res = run_bass_kernel_spmd(nc, in_maps, core_ids=list(range(N_CORES)),
                               trace=trace)
    kernel.last_results = res
    out = np.concatenate([res.results[c]["out"] for c in range(N_CORES)], axis=0)
    return np.ascontiguousarray(out.reshape(b, nb, 512))
